# revision 17
# baseline (speedup 1.0000x reference)
"""Trainium2 Bass kernel: fused MHA (QKV proj -> masked softmax attention -> out proj).

Problem shapes: B=2, T=2048, E=1024, H=16, D=64.
Returns (out [B,T,E], attn [B,H,T,T]) matching the reference.

Sharding: heads across the 8 cores (2 heads/core, both batches on every core).
Each core computes, for its 2 heads:
  - QKV projection in transposed layout (qhT/khT [D,T] per head, vT [2D,T])
  - scores twice on the tensor engine (fp32r full-rate):
      layout B [keys, q] -> exp -> U_B -> attnV matmuls (keys on partitions)
      layout A [q, keys] -> exp (+row-sum Z via accum_out) -> normalize -> attn out
  - masking folded into the matmuls (aug ones/mask contraction row in layout A,
    per-partition activation bias in layout B)
  - per-head out-projection partials, normalized by 1/Z, summed on host.
"""

import numpy as np

B, T, E, H, D = 2, 2048, 1024, 16, 64
NCORES = 8
HPC = H // NCORES  # heads per core

MASK_ROW = -80000.0  # added pre-scale (x0.125 -> -1e4 -> exp == 0.0 in fp32)
MASK_COL = -10000.0  # added post-scale as activation bias


def _col_chunks(total, width):
    out = []
    o = 0
    while o < total:
        w = min(width, total - o)
        out.append((o, w))
        o += w
    return out


def build_nc(T_=T):
    """Build + compile the Bass module (same program on all cores; per-core data
    differs only through the input tensors)."""
    from contextlib import ExitStack

    import concourse.tile as tile
    from concourse import bacc, mybir
    from concourse.masks import make_identity

    f32 = mybir.dt.float32
    f32r = mybir.dt.float32r
    Exp = mybir.ActivationFunctionType.Exp

    KS = E // 128          # contraction slices for projection
    TC = T_ // 128         # 128-row chunks of T
    KCN = T_ // 128        # 128-wide key chunks
    CW = 1024 if T_ % 1024 == 0 else T_   # column-group width for exp ops
    NCG = T_ // CW

    nc = bacc.Bacc(
        "TRN2",
        target_bir_lowering=False,
        debug=False,
        enable_asserts=False,
        num_devices=NCORES,
    )

    qT_d = nc.dram_tensor("qT", (B, E, T_), f32r, kind="ExternalInput").ap()
    wq_d = nc.dram_tensor("wq", (E, 128), f32r, kind="ExternalInput").ap()
    wk_d = nc.dram_tensor("wk", (E, 128), f32r, kind="ExternalInput").ap()
    wv_d = nc.dram_tensor("wv", (E, 128), f32r, kind="ExternalInput").ap()
    bq_d = nc.dram_tensor("bq", (128, 1), f32, kind="ExternalInput").ap()
    bk_d = nc.dram_tensor("bk", (128, 1), f32, kind="ExternalInput").ap()
    bv_d = nc.dram_tensor("bv", (128, 1), f32, kind="ExternalInput").ap()
    mrow_d = nc.dram_tensor("mrow", (B, T_), f32r, kind="ExternalInput").ap()
    mcol_d = nc.dram_tensor("mcol", (128, B, KCN), f32, kind="ExternalInput").ap()
    wout_d = nc.dram_tensor("wout", (64, 2, E), f32r, kind="ExternalInput").ap()
    attn_d = nc.dram_tensor("attn", (B, HPC, T_, T_), f32, kind="ExternalOutput").ap()
    outp_d = nc.dram_tensor("outp", (B, T_, E), f32, kind="ExternalOutput").ap()

    def r(ap):  # operands already declared fp32r
        return ap

    with tile.TileContext(nc) as tc, ExitStack() as ctx:
        consts = ctx.enter_context(tc.tile_pool(name="consts", bufs=1))
        qpool = ctx.enter_context(tc.tile_pool(name="qts", bufs=4))
        sc = ctx.enter_context(tc.tile_pool(name="sc", bufs=2, space="PSUM"))
        pp = ctx.enter_context(tc.tile_pool(name="pp", bufs=2, space="PSUM"))
        po = ctx.enter_context(tc.tile_pool(name="po", bufs=1, space="PSUM"))
        ubp = ctx.enter_context(tc.tile_pool(name="ub", bufs=2))
        atp = ctx.enter_context(tc.tile_pool(name="at", bufs=3))
        hb = ctx.enter_context(tc.tile_pool(name="hb", bufs=1))   # per-b persistents
        zp_pool = ctx.enter_context(tc.tile_pool(name="zs", bufs=4))
        opool = ctx.enter_context(tc.tile_pool(name="op", bufs=3))
        tpool = ctx.enter_context(tc.tile_pool(name="tp", bufs=2))

        # ---- constants ----
        wq_sb = consts.tile([128, KS, 128], f32r, tag="wq")
        wk_sb = consts.tile([128, KS, 128], f32r, tag="wk")
        wv_sb = consts.tile([128, KS, 128], f32r, tag="wv")
        nc.sync.dma_start(wq_sb, wq_d.rearrange("(s p) m -> p s m", p=128))
        nc.sync.dma_start(wk_sb, wk_d.rearrange("(s p) m -> p s m", p=128))
        nc.sync.dma_start(wv_sb, wv_d.rearrange("(s p) m -> p s m", p=128))
        bq_sb = consts.tile([128, 1], f32, tag="bq")
        bk_sb = consts.tile([128, 1], f32, tag="bk")
        bv_sb = consts.tile([128, 1], f32, tag="bv")
        nc.sync.dma_start(bq_sb, bq_d)
        nc.sync.dma_start(bk_sb, bk_d)
        nc.sync.dma_start(bv_sb, bv_d)
        mcol_sb = consts.tile([128, B, KCN], f32, tag="mcol")
        nc.sync.dma_start(mcol_sb, mcol_d)
        wout_sb = consts.tile([64, 2, E], f32r, tag="wout")
        nc.sync.dma_start(wout_sb, wout_d)
        ident0 = consts.tile([128, 128], f32, tag="ident0")
        make_identity(nc, ident0)
        ident = consts.tile([128, 128], f32r, tag="ident")
        nc.vector.tensor_copy(ident, ident0)
        ones0 = consts.tile([1, T_], f32, tag="ones0")
        nc.vector.memset(ones0, 1.0)

        for b in range(B):
            # ---- per-b persistent tiles ----
            qA = [hb.tile([65, T_], f32r, tag=f"qA{hh}", name=f"qA{hh}") for hh in range(2)]
            kA = [hb.tile([65, T_], f32r, tag=f"kA{hh}", name=f"kA{hh}") for hh in range(2)]
            vT_sb = hb.tile([128, T_], f32r, tag="vT")
            vh = hb.tile([128, KCN, 128], f32r, tag="vh")
            outT = [hb.tile([64, T_], f32r, tag=f"oT{hh}", name=f"oT{hh}") for hh in range(2)]
            for hh in range(2):
                nc.vector.tensor_copy(qA[hh][64:65, :], ones0)
                nc.sync.dma_start(kA[hh][64:65, :], mrow_d[b : b + 1, :])

            # ---- projection: qhT/khT/vT = W.T @ q[b].T (+bias) ----
            # pass-sequential (one psum slot at a time) so it overlaps the
            # previous batch's attention phases instead of hogging PSUM.
            for tco, tcw in _col_chunks(T_, 512):
                tsl = slice(tco, tco + tcw)
                qts_l = []
                for ks in range(KS):
                    qts = qpool.tile(
                        [128, 512], f32r, tag="qt", name="qts", bufs=10
                    )
                    nc.sync.dma_start(
                        qts[:, :tcw], qT_d[b, ks * 128 : (ks + 1) * 128, tsl]
                    )
                    qts_l.append(qts)
                for wsb, dst, bias in (
                    (wq_sb, qA, bq_sb),
                    (wk_sb, kA, bk_sb),
                    (wv_sb, None, bv_sb),
                ):
                    ps = pp.tile([128, 512], f32, tag="pp", name="psproj")
                    for ks in range(KS):
                        nc.tensor.matmul(
                            ps[:, :tcw],
                            lhsT=r(wsb[:, ks, :]),
                            rhs=r(qts_l[ks][:, :tcw]),
                            start=(ks == 0),
                            stop=(ks == KS - 1),
                            skip_group_check=True,
                        )
                    if dst is None:
                        nc.vector.tensor_scalar_add(
                            vT_sb[:, tsl], ps[:, :tcw], bias[:, 0:1]
                        )
                    else:
                        nc.vector.tensor_scalar_add(
                            dst[0][0:64, tsl], ps[0:64, :tcw], bias[0:64, 0:1]
                        )
                        shtmp = tpool.tile(
                            [128, 512], f32r, tag="sh", name="shtmp"
                        )
                        nc.vector.tensor_scalar_add(
                            shtmp[64:128, :tcw], ps[64:128, :tcw], bias[64:128, 0:1]
                        )
                        nc.sync.dma_start(dst[1][0:64, tsl], shtmp[64:128, :tcw])

            # ---- vh = vT.T per 128-chunk (PE transpose) ----
            for kc in range(KCN):
                pst = pp.tile([128, 128], f32r, tag="pp", name="pst")
                nc.tensor.transpose(pst, vT_sb[:, kc * 128 : (kc + 1) * 128], ident)
                nc.vector.tensor_copy(vh[:, kc, :], pst)

            rz_h = [None, None]

            def phaseAB(hh):
                """Interleaved emission of phase A (attn rows: DMA-heavy) and
                phase B (U_B @ V: ACT/PE-heavy) so ACT and DMA co-saturate."""
                zps = zp_pool.tile(
                    [128, TC, NCG], f32, tag=f"zp{hh}", name=f"zp{hh}", bufs=2
                )
                rz = zp_pool.tile(
                    [128, TC], f32, tag=f"rz{hh}", name=f"rz{hh}", bufs=2
                )
                rz_h[hh] = rz
                otp_l = [None] * NCG

                def b_unit(j):
                    half, kc = divmod(j, KCN)
                    if kc == 0:
                        otp_l[half] = po.tile([64, CW], f32, tag="po", name="otp")
                    otp = otp_l[half]
                    scb = sc.tile([128, CW], f32, tag="sc", name="scb")
                    for o, w in _col_chunks(CW, 512):
                        nc.tensor.matmul(
                            scb[:, o : o + w],
                            lhsT=r(kA[hh][0:64, kc * 128 : (kc + 1) * 128]),
                            rhs=r(qA[hh][0:64, half * CW + o : half * CW + o + w]),
                            start=True,
                            stop=True,
                        )
                    ubt = ubp.tile([128, CW], f32r, tag="ub", name="ubt")
                    nc.scalar.activation(
                        ubt, scb, Exp, bias=mcol_sb[:, b, kc : kc + 1], scale=0.125
                    )
                    for o, w in _col_chunks(CW, 512):
                        nc.tensor.matmul(
                            otp[:, o : o + w],
                            lhsT=r(vh[:, kc, hh * 64 : (hh + 1) * 64]),
                            rhs=r(ubt[:, o : o + w]),
                            start=(kc == 0),
                            stop=(kc == KCN - 1),
                            skip_group_check=True,
                        )
                    if kc == KCN - 1:
                        nc.vector.tensor_copy(
                            outT[hh][:, half * CW : (half + 1) * CW], otp
                        )

                def a_unit(tci):
                    tsl = slice(tci * 128, (tci + 1) * 128)
                    att = atp.tile([128, T_], f32, tag="at", name="att")
                    for cg in range(NCG):
                        sca = sc.tile([128, CW], f32, tag="sc", name="sca")
                        for o, w in _col_chunks(CW, 512):
                            nc.tensor.matmul(
                                sca[:, o : o + w],
                                lhsT=r(qA[hh][:, tsl]),
                                rhs=r(kA[hh][:, cg * CW + o : cg * CW + o + w]),
                                start=True,
                                stop=True,
                            )
                        nc.scalar.activation(
                            att[:, cg * CW : (cg + 1) * CW],
                            sca,
                            Exp,
                            scale=0.125,
                            accum_out=zps[:, tci, cg : cg + 1],
                        )
                    ztmp = zp_pool.tile([128, 1], f32, tag="ztmp", name="ztmp", bufs=3)
                    nc.vector.tensor_reduce(
                        ztmp,
                        zps[:, tci, :],
                        axis=mybir.AxisListType.X,
                        op=mybir.AluOpType.add,
                    )
                    nc.vector.reciprocal(rz[:, tci : tci + 1], ztmp)
                    nc.vector.tensor_scalar_mul(att, att, rz[:, tci : tci + 1])
                    nc.sync.dma_start(attn_d[b, hh, tsl, :], att)

                nb = NCG * KCN
                jprev = 0
                for tci in range(TC):
                    a_unit(tci)
                    jnext = (tci + 1) * nb // TC
                    for j in range(jprev, jnext):
                        b_unit(j)
                    jprev = jnext

            def outproj():
                for tci in range(TC):
                    tsl = slice(tci * 128, (tci + 1) * 128)
                    pss = []
                    for hh in range(2):
                        ps = sc.tile([128, E], f32, tag="sc", name="psop")
                        for eo, ew in _col_chunks(E, 512):
                            nc.tensor.matmul(
                                ps[:, eo : eo + ew],
                                lhsT=r(outT[hh][:, tsl]),
                                rhs=r(wout_sb[:, hh, eo : eo + ew]),
                                start=True,
                                stop=True,
                            )
                        pss.append(ps)
                    t0 = tpool.tile([128, E], f32, tag="tp", name="t0")
                    nc.scalar.mul(t0, pss[0], rz_h[0][:, tci : tci + 1])
                    ot = opool.tile([128, E], f32, tag="op", name="ot")
                    nc.vector.tensor_scalar_mul(
                        ot, pss[1], rz_h[1][:, tci : tci + 1]
                    )
                    nc.vector.tensor_add(ot, ot, t0)
                    nc.sync.dma_start(outp_d[b, tsl, :], ot)

            phaseAB(0)
            phaseAB(1)
            outproj()


    nc.compile()
    return nc


def make_in_maps(q, mask, Wqkv, bqkv, Wout, T_=T):
    """Per-core input dicts (host-side sharding by heads)."""
    q = np.ascontiguousarray(np.asarray(q, dtype=np.float32))
    mask = np.asarray(mask).astype(bool)
    Wqkv = np.asarray(Wqkv, dtype=np.float32)
    bqkv = np.asarray(bqkv, dtype=np.float32)
    Wout = np.asarray(Wout, dtype=np.float32)

    KCN = T_ // 128
    qT = np.ascontiguousarray(q.transpose(0, 2, 1))  # [B, E, T]
    mrow = np.where(mask, np.float32(MASK_ROW), np.float32(0.0)).astype(np.float32)
    mcolv = np.where(mask, np.float32(MASK_COL), np.float32(0.0)).astype(np.float32)
    mcol = np.ascontiguousarray(
        mcolv.reshape(B, KCN, 128).transpose(2, 0, 1)
    )  # [128, B, KCN]

    in_maps = []
    for c in range(NCORES):
        hs = [HPC * c + i for i in range(HPC)]
        wq = np.concatenate([Wqkv[:, h * D : (h + 1) * D] for h in hs], axis=1)
        wk = np.concatenate(
            [Wqkv[:, E + h * D : E + (h + 1) * D] for h in hs], axis=1
        )
        wv = np.concatenate(
            [Wqkv[:, 2 * E + h * D : 2 * E + (h + 1) * D] for h in hs], axis=1
        )
        bq = np.concatenate([bqkv[h * D : (h + 1) * D] for h in hs]).reshape(128, 1)
        bk = np.concatenate(
            [bqkv[E + h * D : E + (h + 1) * D] for h in hs]
        ).reshape(128, 1)
        bv = np.concatenate([bqkv[2 * E + h * D : 2 * E + (h + 1) * D] for h in hs])
        wout = np.stack([Wout[h * D : (h + 1) * D, :] for h in hs], axis=1)
        in_maps.append(
            dict(
                qT=qT,
                wq=np.ascontiguousarray(wq),
                wk=np.ascontiguousarray(wk),
                wv=np.ascontiguousarray(wv),
                bq=np.ascontiguousarray(bq),
                bk=np.ascontiguousarray(bk),
                bv=np.ascontiguousarray(bv.reshape(128, 1)),
                mrow=mrow,
                mcol=mcol,
                wout=np.ascontiguousarray(wout),
            )
        )
    return in_maps


def assemble(results, bout):
    """Combine per-core results into (out, attn)."""
    bout = np.asarray(bout, dtype=np.float32)
    T_ = results[0]["outp"].shape[1]
    attn = np.empty((B, H, T_, T_), dtype=np.float32)
    out = np.zeros((B, T_, E), dtype=np.float32)
    for c, res in enumerate(results):
        a = res["attn"]  # [B, HPC, T, T]
        for i in range(HPC):
            attn[:, HPC * c + i] = a[:, i]
        out += res["outp"]
    out += bout
    return out, attn


_CACHE = {}


def _get_nc():
    if "nc" not in _CACHE:
        _CACHE["nc"] = build_nc(T)
    return _CACHE["nc"]


def kernel(q, k=None, v=None, mask=None, Wqkv=None, bqkv=None, Wout=None, bout=None):
    """Full-input entry point. k/v are unused (reference derives qkv from q)."""
    from concourse.bass_utils import run_bass_kernel_spmd

    nc = _get_nc()
    in_maps = make_in_maps(q, mask, Wqkv, bqkv, Wout, T_=T)
    res = run_bass_kernel_spmd(nc, in_maps, core_ids=list(range(NCORES)))
    return assemble(res.results, bout)


# revision 22
# speedup vs baseline: 1.0165x; 1.0165x over previous
"""Trainium2 Bass kernel: fused MHA (QKV proj -> masked softmax attention -> out proj).

Problem shapes: B=2, T=2048, E=1024, H=16, D=64.
Returns (out [B,T,E], attn [B,H,T,T]) matching the reference.

Sharding: heads across the 8 cores (2 heads/core, both batches on every core).
Each core computes, for its 2 heads:
  - QKV projection in transposed layout (qhT/khT [D,T] per head, vT [2D,T])
  - scores twice on the tensor engine (fp32r full-rate):
      layout B [keys, q] -> exp -> U_B -> attnV matmuls (keys on partitions)
      layout A [q, keys] -> exp (+row-sum Z via accum_out) -> normalize -> attn out
  - masking folded into the matmuls (aug ones/mask contraction row in layout A,
    per-partition activation bias in layout B)
  - per-head out-projection partials, normalized by 1/Z, summed on host.
"""

import numpy as np

B, T, E, H, D = 2, 2048, 1024, 16, 64
NCORES = 8
HPC = H // NCORES  # heads per core

MASK_ROW = -80000.0  # added pre-scale (x0.125 -> -1e4 -> exp == 0.0 in fp32)
MASK_COL = -10000.0  # added post-scale as activation bias


def _col_chunks(total, width):
    out = []
    o = 0
    while o < total:
        w = min(width, total - o)
        out.append((o, w))
        o += w
    return out


def build_nc(T_=T):
    """Build + compile the Bass module (same program on all cores; per-core data
    differs only through the input tensors)."""
    from contextlib import ExitStack

    import concourse.tile as tile
    from concourse import bacc, mybir
    from concourse.masks import make_identity

    f32 = mybir.dt.float32
    f32r = mybir.dt.float32r
    Exp = mybir.ActivationFunctionType.Exp

    KS = E // 128          # contraction slices for projection
    TC = T_ // 128         # 128-row chunks of T
    KCN = T_ // 128        # 128-wide key chunks
    CW = 1024 if T_ % 1024 == 0 else T_   # column-group width for exp ops
    NCG = T_ // CW

    nc = bacc.Bacc(
        "TRN2",
        target_bir_lowering=False,
        debug=False,
        enable_asserts=False,
        num_devices=NCORES,
    )

    qT_d = nc.dram_tensor("qT", (B, E, T_), f32r, kind="ExternalInput").ap()
    wq_d = nc.dram_tensor("wq", (E, 128), f32r, kind="ExternalInput").ap()
    wk_d = nc.dram_tensor("wk", (E, 128), f32r, kind="ExternalInput").ap()
    wv_d = nc.dram_tensor("wv", (E, 128), f32r, kind="ExternalInput").ap()
    bq_d = nc.dram_tensor("bq", (128, 1), f32, kind="ExternalInput").ap()
    bk_d = nc.dram_tensor("bk", (128, 1), f32, kind="ExternalInput").ap()
    bv_d = nc.dram_tensor("bv", (128, 1), f32, kind="ExternalInput").ap()
    mrow_d = nc.dram_tensor("mrow", (B, T_), f32r, kind="ExternalInput").ap()
    mcol_d = nc.dram_tensor("mcol", (128, B, KCN), f32, kind="ExternalInput").ap()
    wout_d = nc.dram_tensor("wout", (64, 2, E), f32r, kind="ExternalInput").ap()
    attn_d = nc.dram_tensor("attn", (B, HPC, T_, T_), f32, kind="ExternalOutput").ap()
    outp_d = nc.dram_tensor("outp", (B, T_, E), f32, kind="ExternalOutput").ap()

    def r(ap):  # operands already declared fp32r
        return ap

    with tile.TileContext(nc) as tc, ExitStack() as ctx:
        consts = ctx.enter_context(tc.tile_pool(name="consts", bufs=1))
        qpool = ctx.enter_context(tc.tile_pool(name="qts", bufs=4))
        sc = ctx.enter_context(tc.tile_pool(name="sc", bufs=2, space="PSUM"))
        pp = ctx.enter_context(tc.tile_pool(name="pp", bufs=2, space="PSUM"))
        po = ctx.enter_context(tc.tile_pool(name="po", bufs=1, space="PSUM"))
        ubp = ctx.enter_context(tc.tile_pool(name="ub", bufs=3))
        atp = ctx.enter_context(tc.tile_pool(name="at", bufs=4))
        hb = ctx.enter_context(tc.tile_pool(name="hb", bufs=1))   # per-b persistents
        zp_pool = ctx.enter_context(tc.tile_pool(name="zs", bufs=4))
        opool = ctx.enter_context(tc.tile_pool(name="op", bufs=3))
        tpool = ctx.enter_context(tc.tile_pool(name="tp", bufs=2))

        # ---- constants ----
        wq_sb = consts.tile([128, KS, 128], f32r, tag="wq")
        wk_sb = consts.tile([128, KS, 128], f32r, tag="wk")
        wv_sb = consts.tile([128, KS, 128], f32r, tag="wv")
        nc.sync.dma_start(wq_sb, wq_d.rearrange("(s p) m -> p s m", p=128))
        nc.sync.dma_start(wk_sb, wk_d.rearrange("(s p) m -> p s m", p=128))
        nc.sync.dma_start(wv_sb, wv_d.rearrange("(s p) m -> p s m", p=128))
        bq_sb = consts.tile([128, 1], f32, tag="bq")
        bk_sb = consts.tile([128, 1], f32, tag="bk")
        bv_sb = consts.tile([128, 1], f32, tag="bv")
        nc.sync.dma_start(bq_sb, bq_d)
        nc.sync.dma_start(bk_sb, bk_d)
        nc.sync.dma_start(bv_sb, bv_d)
        mcol_sb = consts.tile([128, B, KCN], f32, tag="mcol")
        nc.sync.dma_start(mcol_sb, mcol_d)
        wout_sb = consts.tile([64, 2, E], f32r, tag="wout")
        nc.sync.dma_start(wout_sb, wout_d)
        ident0 = consts.tile([128, 128], f32, tag="ident0")
        make_identity(nc, ident0)
        ident = consts.tile([128, 128], f32r, tag="ident")
        nc.vector.tensor_copy(ident, ident0)
        ones0 = consts.tile([1, T_], f32, tag="ones0")
        nc.vector.memset(ones0, 1.0)

        for b in range(B):
            # ---- per-b persistent tiles ----
            qA = [hb.tile([65, T_], f32r, tag=f"qA{hh}", name=f"qA{hh}") for hh in range(2)]
            kA = [hb.tile([65, T_], f32r, tag=f"kA{hh}", name=f"kA{hh}") for hh in range(2)]
            vT_sb = hb.tile([128, T_], f32r, tag="vT")
            vh = hb.tile([128, KCN, 128], f32r, tag="vh")
            outT = [hb.tile([64, T_], f32r, tag=f"oT{hh}", name=f"oT{hh}") for hh in range(2)]
            for hh in range(2):
                nc.vector.tensor_copy(qA[hh][64:65, :], ones0)
                nc.sync.dma_start(kA[hh][64:65, :], mrow_d[b : b + 1, :])

            # ---- projection: qhT/khT/vT = W.T @ q[b].T (+bias) ----
            # pass-sequential (one psum slot at a time) so it overlaps the
            # previous batch's attention phases instead of hogging PSUM.
            for tco, tcw in _col_chunks(T_, 512):
                tsl = slice(tco, tco + tcw)
                qts_l = []
                for ks in range(KS):
                    qts = qpool.tile(
                        [128, 512], f32r, tag="qt", name="qts", bufs=12
                    )
                    nc.sync.dma_start(
                        qts[:, :tcw], qT_d[b, ks * 128 : (ks + 1) * 128, tsl]
                    )
                    qts_l.append(qts)
                for wsb, dst, bias in (
                    (wq_sb, qA, bq_sb),
                    (wk_sb, kA, bk_sb),
                    (wv_sb, None, bv_sb),
                ):
                    ps = pp.tile([128, 512], f32, tag="pp", name="psproj")
                    for ks in range(KS):
                        nc.tensor.matmul(
                            ps[:, :tcw],
                            lhsT=r(wsb[:, ks, :]),
                            rhs=r(qts_l[ks][:, :tcw]),
                            start=(ks == 0),
                            stop=(ks == KS - 1),
                            skip_group_check=True,
                        )
                    if dst is None:
                        nc.vector.tensor_scalar_add(
                            vT_sb[:, tsl], ps[:, :tcw], bias[:, 0:1]
                        )
                    else:
                        nc.vector.tensor_scalar_add(
                            dst[0][0:64, tsl], ps[0:64, :tcw], bias[0:64, 0:1]
                        )
                        shtmp = tpool.tile(
                            [128, 512], f32r, tag="sh", name="shtmp"
                        )
                        nc.vector.tensor_scalar_add(
                            shtmp[64:128, :tcw], ps[64:128, :tcw], bias[64:128, 0:1]
                        )
                        nc.sync.dma_start(dst[1][0:64, tsl], shtmp[64:128, :tcw])

            # ---- vh = vT.T per 128-chunk (PE transpose) ----
            for kc in range(KCN):
                pst = pp.tile([128, 128], f32r, tag="pp", name="pst")
                nc.tensor.transpose(pst, vT_sb[:, kc * 128 : (kc + 1) * 128], ident)
                nc.vector.tensor_copy(vh[:, kc, :], pst)

            rz_h = [None, None]

            def phaseAB(hh, tail_units=None):
                """Interleaved emission of phase A (attn rows: DMA-heavy) and
                phase B (U_B @ V: ACT/PE-heavy) so ACT and DMA co-saturate."""
                zps = zp_pool.tile(
                    [128, TC, NCG], f32, tag=f"zp{hh}", name=f"zp{hh}", bufs=2
                )
                rz = zp_pool.tile(
                    [128, TC], f32, tag=f"rz{hh}", name=f"rz{hh}", bufs=2
                )
                rz_h[hh] = rz
                otp_l = [None] * NCG

                def b_unit(j):
                    half, kc = divmod(j, KCN)
                    if kc == 0:
                        otp_l[half] = po.tile([64, CW], f32, tag="po", name="otp")
                    otp = otp_l[half]
                    scb = sc.tile([128, CW], f32, tag="sc", name="scb")
                    for o, w in _col_chunks(CW, 512):
                        nc.tensor.matmul(
                            scb[:, o : o + w],
                            lhsT=r(kA[hh][0:64, kc * 128 : (kc + 1) * 128]),
                            rhs=r(qA[hh][0:64, half * CW + o : half * CW + o + w]),
                            start=True,
                            stop=True,
                        )
                    ubt = ubp.tile([128, CW], f32r, tag="ub", name="ubt")
                    nc.scalar.activation(
                        ubt, scb, Exp, bias=mcol_sb[:, b, kc : kc + 1], scale=0.125
                    )
                    for o, w in _col_chunks(CW, 512):
                        nc.tensor.matmul(
                            otp[:, o : o + w],
                            lhsT=r(vh[:, kc, hh * 64 : (hh + 1) * 64]),
                            rhs=r(ubt[:, o : o + w]),
                            start=(kc == 0),
                            stop=(kc == KCN - 1),
                            skip_group_check=True,
                        )
                    if kc == KCN - 1:
                        nc.vector.tensor_copy(
                            outT[hh][:, half * CW : (half + 1) * CW], otp
                        )

                def a_unit(tci):
                    tsl = slice(tci * 128, (tci + 1) * 128)
                    att = atp.tile([128, T_], f32, tag="at", name="att")
                    for cg in range(NCG):
                        sca = sc.tile([128, CW], f32, tag="sc", name="sca")
                        for o, w in _col_chunks(CW, 512):
                            nc.tensor.matmul(
                                sca[:, o : o + w],
                                lhsT=r(qA[hh][:, tsl]),
                                rhs=r(kA[hh][:, cg * CW + o : cg * CW + o + w]),
                                start=True,
                                stop=True,
                            )
                        nc.scalar.activation(
                            att[:, cg * CW : (cg + 1) * CW],
                            sca,
                            Exp,
                            scale=0.125,
                            accum_out=zps[:, tci, cg : cg + 1],
                        )
                    ztmp = zp_pool.tile([128, 1], f32, tag="ztmp", name="ztmp", bufs=3)
                    nc.vector.tensor_reduce(
                        ztmp,
                        zps[:, tci, :],
                        axis=mybir.AxisListType.X,
                        op=mybir.AluOpType.add,
                    )
                    nc.vector.reciprocal(rz[:, tci : tci + 1], ztmp)
                    nc.vector.tensor_scalar_mul(att, att, rz[:, tci : tci + 1])
                    nc.sync.dma_start(attn_d[b, hh, tsl, :], att)

                nb = NCG * KCN
                jprev = 0
                for tci in range(TC):
                    a_unit(tci)
                    jnext = (tci + 1) * nb // TC
                    for j in range(jprev, jnext):
                        b_unit(j)
                        if tail_units is not None and (j + 1) % KCN == 0:
                            # a half of outT just completed; emit tail work
                            # covering that column range (overlaps remaining
                            # phase work instead of serializing at the end)
                            half = (j + 1) // KCN - 1
                            lo = half * (CW // 128)
                            hi = (half + 1) * (CW // 128)
                            for u in tail_units[lo:hi]:
                                u()
                    jprev = jnext

            def outproj():
                for tci in range(TC):
                    tsl = slice(tci * 128, (tci + 1) * 128)
                    pss = []
                    for hh in range(2):
                        ps = sc.tile([128, E], f32, tag="sc", name="psop")
                        for eo, ew in _col_chunks(E, 512):
                            nc.tensor.matmul(
                                ps[:, eo : eo + ew],
                                lhsT=r(outT[hh][:, tsl]),
                                rhs=r(wout_sb[:, hh, eo : eo + ew]),
                                start=True,
                                stop=True,
                            )
                        pss.append(ps)
                    t0 = tpool.tile([128, E], f32, tag="tp", name="t0")
                    nc.scalar.mul(t0, pss[0], rz_h[0][:, tci : tci + 1])
                    ot = opool.tile([128, E], f32, tag="op", name="ot")
                    nc.vector.tensor_scalar_mul(
                        ot, pss[1], rz_h[1][:, tci : tci + 1]
                    )
                    nc.vector.tensor_add(ot, ot, t0)
                    nc.sync.dma_start(outp_d[b, tsl, :], ot)

            phaseAB(0)
            phaseAB(1)
            outproj()


    nc.compile()
    return nc


def make_in_maps(q, mask, Wqkv, bqkv, Wout, T_=T):
    """Per-core input dicts (host-side sharding by heads)."""
    q = np.ascontiguousarray(np.asarray(q, dtype=np.float32))
    mask = np.asarray(mask).astype(bool)
    Wqkv = np.asarray(Wqkv, dtype=np.float32)
    bqkv = np.asarray(bqkv, dtype=np.float32)
    Wout = np.asarray(Wout, dtype=np.float32)

    KCN = T_ // 128
    qT = np.ascontiguousarray(q.transpose(0, 2, 1))  # [B, E, T]
    mrow = np.where(mask, np.float32(MASK_ROW), np.float32(0.0)).astype(np.float32)
    mcolv = np.where(mask, np.float32(MASK_COL), np.float32(0.0)).astype(np.float32)
    mcol = np.ascontiguousarray(
        mcolv.reshape(B, KCN, 128).transpose(2, 0, 1)
    )  # [128, B, KCN]

    in_maps = []
    for c in range(NCORES):
        hs = [HPC * c + i for i in range(HPC)]
        wq = np.concatenate([Wqkv[:, h * D : (h + 1) * D] for h in hs], axis=1)
        wk = np.concatenate(
            [Wqkv[:, E + h * D : E + (h + 1) * D] for h in hs], axis=1
        )
        wv = np.concatenate(
            [Wqkv[:, 2 * E + h * D : 2 * E + (h + 1) * D] for h in hs], axis=1
        )
        bq = np.concatenate([bqkv[h * D : (h + 1) * D] for h in hs]).reshape(128, 1)
        bk = np.concatenate(
            [bqkv[E + h * D : E + (h + 1) * D] for h in hs]
        ).reshape(128, 1)
        bv = np.concatenate([bqkv[2 * E + h * D : 2 * E + (h + 1) * D] for h in hs])
        wout = np.stack([Wout[h * D : (h + 1) * D, :] for h in hs], axis=1)
        in_maps.append(
            dict(
                qT=qT,
                wq=np.ascontiguousarray(wq),
                wk=np.ascontiguousarray(wk),
                wv=np.ascontiguousarray(wv),
                bq=np.ascontiguousarray(bq),
                bk=np.ascontiguousarray(bk),
                bv=np.ascontiguousarray(bv.reshape(128, 1)),
                mrow=mrow,
                mcol=mcol,
                wout=np.ascontiguousarray(wout),
            )
        )
    return in_maps


def assemble(results, bout):
    """Combine per-core results into (out, attn)."""
    bout = np.asarray(bout, dtype=np.float32)
    T_ = results[0]["outp"].shape[1]
    attn = np.empty((B, H, T_, T_), dtype=np.float32)
    out = np.zeros((B, T_, E), dtype=np.float32)
    for c, res in enumerate(results):
        a = res["attn"]  # [B, HPC, T, T]
        for i in range(HPC):
            attn[:, HPC * c + i] = a[:, i]
        out += res["outp"]
    out += bout
    return out, attn


_CACHE = {}


def _get_nc():
    if "nc" not in _CACHE:
        _CACHE["nc"] = build_nc(T)
    return _CACHE["nc"]


def kernel(q, k=None, v=None, mask=None, Wqkv=None, bqkv=None, Wout=None, bout=None):
    """Full-input entry point. k/v are unused (reference derives qkv from q)."""
    from concourse.bass_utils import run_bass_kernel_spmd

    nc = _get_nc()
    in_maps = make_in_maps(q, mask, Wqkv, bqkv, Wout, T_=T)
    res = run_bass_kernel_spmd(nc, in_maps, core_ids=list(range(NCORES)))
    return assemble(res.results, bout)


# revision 23
# speedup vs baseline: 1.0236x; 1.0070x over previous
"""Trainium2 Bass kernel: fused MHA (QKV proj -> masked softmax attention -> out proj).

Problem shapes: B=2, T=2048, E=1024, H=16, D=64.
Returns (out [B,T,E], attn [B,H,T,T]) matching the reference.

Sharding: heads across the 8 cores (2 heads/core, both batches on every core).
Each core computes, for its 2 heads:
  - QKV projection in transposed layout (qhT/khT [D,T] per head, vT [2D,T])
  - scores twice on the tensor engine (fp32r full-rate):
      layout B [keys, q] -> exp -> U_B -> attnV matmuls (keys on partitions)
      layout A [q, keys] -> exp (+row-sum Z via accum_out) -> normalize -> attn out
  - masking folded into the matmuls (aug ones/mask contraction row in layout A,
    per-partition activation bias in layout B)
  - per-head out-projection partials, normalized by 1/Z, summed on host.
"""

import numpy as np

B, T, E, H, D = 2, 2048, 1024, 16, 64
NCORES = 8
HPC = H // NCORES  # heads per core

MASK_ROW = -80000.0  # added pre-scale (x0.125 -> -1e4 -> exp == 0.0 in fp32)
MASK_COL = -10000.0  # added post-scale as activation bias


def _col_chunks(total, width):
    out = []
    o = 0
    while o < total:
        w = min(width, total - o)
        out.append((o, w))
        o += w
    return out


def build_nc(T_=T):
    """Build + compile the Bass module (same program on all cores; per-core data
    differs only through the input tensors)."""
    from contextlib import ExitStack

    import concourse.tile as tile
    from concourse import bacc, mybir
    from concourse.masks import make_identity

    f32 = mybir.dt.float32
    f32r = mybir.dt.float32r
    Exp = mybir.ActivationFunctionType.Exp

    KS = E // 128          # contraction slices for projection
    TC = T_ // 128         # 128-row chunks of T
    KCN = T_ // 128        # 128-wide key chunks
    CW = 1024 if T_ % 1024 == 0 else T_   # column-group width for exp ops
    NCG = T_ // CW

    nc = bacc.Bacc(
        "TRN2",
        target_bir_lowering=False,
        debug=False,
        enable_asserts=False,
        num_devices=NCORES,
    )

    qT_d = nc.dram_tensor("qT", (B, E, T_), f32r, kind="ExternalInput").ap()
    wq_d = nc.dram_tensor("wq", (E, 128), f32r, kind="ExternalInput").ap()
    wk_d = nc.dram_tensor("wk", (E, 128), f32r, kind="ExternalInput").ap()
    wv_d = nc.dram_tensor("wv", (E, 128), f32r, kind="ExternalInput").ap()
    bq_d = nc.dram_tensor("bq", (128, 1), f32, kind="ExternalInput").ap()
    bk_d = nc.dram_tensor("bk", (128, 1), f32, kind="ExternalInput").ap()
    bv_d = nc.dram_tensor("bv", (128, 1), f32, kind="ExternalInput").ap()
    mrow_d = nc.dram_tensor("mrow", (B, T_), f32r, kind="ExternalInput").ap()
    mcol_d = nc.dram_tensor("mcol", (128, B, KCN), f32, kind="ExternalInput").ap()
    wout_d = nc.dram_tensor("wout", (64, 2, E), f32r, kind="ExternalInput").ap()
    attn_d = nc.dram_tensor("attn", (B, HPC, T_, T_), f32, kind="ExternalOutput").ap()
    outp_d = nc.dram_tensor("outp", (B, T_, E), f32, kind="ExternalOutput").ap()

    def r(ap):  # operands already declared fp32r
        return ap

    with tile.TileContext(nc) as tc, ExitStack() as ctx:
        consts = ctx.enter_context(tc.tile_pool(name="consts", bufs=1))
        qpool = ctx.enter_context(tc.tile_pool(name="qts", bufs=4))
        sc = ctx.enter_context(tc.tile_pool(name="sc", bufs=2, space="PSUM"))
        pp = ctx.enter_context(tc.tile_pool(name="pp", bufs=2, space="PSUM"))
        po = ctx.enter_context(tc.tile_pool(name="po", bufs=1, space="PSUM"))
        ubp = ctx.enter_context(tc.tile_pool(name="ub", bufs=3))
        atp = ctx.enter_context(tc.tile_pool(name="at", bufs=4))
        hb = ctx.enter_context(tc.tile_pool(name="hb", bufs=1))   # per-b persistents
        zp_pool = ctx.enter_context(tc.tile_pool(name="zs", bufs=4))
        opool = ctx.enter_context(tc.tile_pool(name="op", bufs=4))
        tpool = ctx.enter_context(tc.tile_pool(name="tp", bufs=2))

        # ---- constants ----
        wq_sb = consts.tile([128, KS, 128], f32r, tag="wq")
        wk_sb = consts.tile([128, KS, 128], f32r, tag="wk")
        wv_sb = consts.tile([128, KS, 128], f32r, tag="wv")
        nc.sync.dma_start(wq_sb, wq_d.rearrange("(s p) m -> p s m", p=128))
        nc.sync.dma_start(wk_sb, wk_d.rearrange("(s p) m -> p s m", p=128))
        nc.sync.dma_start(wv_sb, wv_d.rearrange("(s p) m -> p s m", p=128))
        bq_sb = consts.tile([128, 1], f32, tag="bq")
        bk_sb = consts.tile([128, 1], f32, tag="bk")
        bv_sb = consts.tile([128, 1], f32, tag="bv")
        nc.sync.dma_start(bq_sb, bq_d)
        nc.sync.dma_start(bk_sb, bk_d)
        nc.sync.dma_start(bv_sb, bv_d)
        mcol_sb = consts.tile([128, B, KCN], f32, tag="mcol")
        nc.sync.dma_start(mcol_sb, mcol_d)
        wout_sb = consts.tile([64, 2, E], f32r, tag="wout")
        nc.sync.dma_start(wout_sb, wout_d)
        ident0 = consts.tile([128, 128], f32, tag="ident0")
        make_identity(nc, ident0)
        ident = consts.tile([128, 128], f32r, tag="ident")
        nc.vector.tensor_copy(ident, ident0)
        ones0 = consts.tile([1, T_], f32, tag="ones0")
        nc.vector.memset(ones0, 1.0)

        for b in range(B):
            # ---- per-b persistent tiles ----
            qA = [hb.tile([65, T_], f32r, tag=f"qA{hh}", name=f"qA{hh}") for hh in range(2)]
            kA = [hb.tile([65, T_], f32r, tag=f"kA{hh}", name=f"kA{hh}") for hh in range(2)]
            vT_sb = hb.tile([128, T_], f32r, tag="vT")
            vh = hb.tile([128, KCN, 128], f32r, tag="vh")
            outT = [hb.tile([64, T_], f32r, tag=f"oT{hh}", name=f"oT{hh}") for hh in range(2)]
            for hh in range(2):
                nc.vector.tensor_copy(qA[hh][64:65, :], ones0)
                nc.sync.dma_start(kA[hh][64:65, :], mrow_d[b : b + 1, :])

            # ---- projection: qhT/khT/vT = W.T @ q[b].T (+bias) ----
            # pass-sequential (one psum slot at a time) so it overlaps the
            # previous batch's attention phases instead of hogging PSUM.
            for tco, tcw in _col_chunks(T_, 512):
                tsl = slice(tco, tco + tcw)
                qts_l = []
                for ks in range(KS):
                    qts = qpool.tile(
                        [128, 512], f32r, tag="qt", name="qts", bufs=16
                    )
                    nc.sync.dma_start(
                        qts[:, :tcw], qT_d[b, ks * 128 : (ks + 1) * 128, tsl]
                    )
                    qts_l.append(qts)
                for wsb, dst, bias in (
                    (wq_sb, qA, bq_sb),
                    (wk_sb, kA, bk_sb),
                    (wv_sb, None, bv_sb),
                ):
                    ps = pp.tile([128, 512], f32, tag="pp", name="psproj")
                    for ks in range(KS):
                        nc.tensor.matmul(
                            ps[:, :tcw],
                            lhsT=r(wsb[:, ks, :]),
                            rhs=r(qts_l[ks][:, :tcw]),
                            start=(ks == 0),
                            stop=(ks == KS - 1),
                            skip_group_check=True,
                        )
                    if dst is None:
                        nc.vector.tensor_scalar_add(
                            vT_sb[:, tsl], ps[:, :tcw], bias[:, 0:1]
                        )
                    else:
                        nc.vector.tensor_scalar_add(
                            dst[0][0:64, tsl], ps[0:64, :tcw], bias[0:64, 0:1]
                        )
                        shtmp = tpool.tile(
                            [128, 512], f32r, tag="sh", name="shtmp"
                        )
                        nc.vector.tensor_scalar_add(
                            shtmp[64:128, :tcw], ps[64:128, :tcw], bias[64:128, 0:1]
                        )
                        nc.sync.dma_start(dst[1][0:64, tsl], shtmp[64:128, :tcw])

            # ---- vh = vT.T per 128-chunk (PE transpose) ----
            for kc in range(KCN):
                pst = pp.tile([128, 128], f32r, tag="pp", name="pst")
                nc.tensor.transpose(pst, vT_sb[:, kc * 128 : (kc + 1) * 128], ident)
                nc.vector.tensor_copy(vh[:, kc, :], pst)

            rz_h = [None, None]

            def phaseAB(hh, tail_units=None):
                """Interleaved emission of phase A (attn rows: DMA-heavy) and
                phase B (U_B @ V: ACT/PE-heavy) so ACT and DMA co-saturate."""
                zps = zp_pool.tile(
                    [128, TC, NCG], f32, tag=f"zp{hh}", name=f"zp{hh}", bufs=2
                )
                rz = zp_pool.tile(
                    [128, TC], f32, tag=f"rz{hh}", name=f"rz{hh}", bufs=2
                )
                rz_h[hh] = rz
                otp_l = [None] * NCG

                def b_unit(j):
                    half, kc = divmod(j, KCN)
                    if kc == 0:
                        otp_l[half] = po.tile([64, CW], f32, tag="po", name="otp")
                    otp = otp_l[half]
                    scb = sc.tile([128, CW], f32, tag="sc", name="scb")
                    for o, w in _col_chunks(CW, 512):
                        nc.tensor.matmul(
                            scb[:, o : o + w],
                            lhsT=r(kA[hh][0:64, kc * 128 : (kc + 1) * 128]),
                            rhs=r(qA[hh][0:64, half * CW + o : half * CW + o + w]),
                            start=True,
                            stop=True,
                        )
                    ubt = ubp.tile([128, CW], f32r, tag="ub", name="ubt")
                    nc.scalar.activation(
                        ubt, scb, Exp, bias=mcol_sb[:, b, kc : kc + 1], scale=0.125
                    )
                    for o, w in _col_chunks(CW, 512):
                        nc.tensor.matmul(
                            otp[:, o : o + w],
                            lhsT=r(vh[:, kc, hh * 64 : (hh + 1) * 64]),
                            rhs=r(ubt[:, o : o + w]),
                            start=(kc == 0),
                            stop=(kc == KCN - 1),
                            skip_group_check=True,
                        )
                    if kc == KCN - 1:
                        nc.vector.tensor_copy(
                            outT[hh][:, half * CW : (half + 1) * CW], otp
                        )

                def a_unit(tci):
                    tsl = slice(tci * 128, (tci + 1) * 128)
                    att = atp.tile([128, T_], f32, tag="at", name="att")
                    for cg in range(NCG):
                        sca = sc.tile([128, CW], f32, tag="sc", name="sca")
                        for o, w in _col_chunks(CW, 512):
                            nc.tensor.matmul(
                                sca[:, o : o + w],
                                lhsT=r(qA[hh][:, tsl]),
                                rhs=r(kA[hh][:, cg * CW + o : cg * CW + o + w]),
                                start=True,
                                stop=True,
                            )
                        nc.scalar.activation(
                            att[:, cg * CW : (cg + 1) * CW],
                            sca,
                            Exp,
                            scale=0.125,
                            accum_out=zps[:, tci, cg : cg + 1],
                        )
                    ztmp = zp_pool.tile([128, 1], f32, tag="ztmp", name="ztmp", bufs=3)
                    nc.vector.tensor_reduce(
                        ztmp,
                        zps[:, tci, :],
                        axis=mybir.AxisListType.X,
                        op=mybir.AluOpType.add,
                    )
                    nc.vector.reciprocal(rz[:, tci : tci + 1], ztmp)
                    nc.vector.tensor_scalar_mul(att, att, rz[:, tci : tci + 1])
                    nc.sync.dma_start(attn_d[b, hh, tsl, :], att)

                nb = NCG * KCN
                jprev = 0
                for tci in range(TC):
                    a_unit(tci)
                    jnext = (tci + 1) * nb // TC
                    for j in range(jprev, jnext):
                        b_unit(j)
                        if tail_units is not None and (j + 1) % KCN == 0:
                            # a half of outT just completed; emit tail work
                            # covering that column range (overlaps remaining
                            # phase work instead of serializing at the end)
                            half = (j + 1) // KCN - 1
                            lo = half * (CW // 128)
                            hi = (half + 1) * (CW // 128)
                            for u in tail_units[lo:hi]:
                                u()
                    jprev = jnext

            def outproj():
                for tci in range(TC):
                    tsl = slice(tci * 128, (tci + 1) * 128)
                    pss = []
                    for hh in range(2):
                        ps = sc.tile([128, E], f32, tag="sc", name="psop")
                        for eo, ew in _col_chunks(E, 512):
                            nc.tensor.matmul(
                                ps[:, eo : eo + ew],
                                lhsT=r(outT[hh][:, tsl]),
                                rhs=r(wout_sb[:, hh, eo : eo + ew]),
                                start=True,
                                stop=True,
                            )
                        pss.append(ps)
                    t0 = tpool.tile([128, E], f32, tag="tp", name="t0")
                    nc.scalar.mul(t0, pss[0], rz_h[0][:, tci : tci + 1])
                    ot = opool.tile([128, E], f32, tag="op", name="ot")
                    nc.vector.tensor_scalar_mul(
                        ot, pss[1], rz_h[1][:, tci : tci + 1]
                    )
                    nc.vector.tensor_add(ot, ot, t0)
                    nc.sync.dma_start(outp_d[b, tsl, :], ot)

            phaseAB(0)
            phaseAB(1)
            outproj()


    nc.compile()
    return nc


def make_in_maps(q, mask, Wqkv, bqkv, Wout, T_=T):
    """Per-core input dicts (host-side sharding by heads)."""
    q = np.ascontiguousarray(np.asarray(q, dtype=np.float32))
    mask = np.asarray(mask).astype(bool)
    Wqkv = np.asarray(Wqkv, dtype=np.float32)
    bqkv = np.asarray(bqkv, dtype=np.float32)
    Wout = np.asarray(Wout, dtype=np.float32)

    KCN = T_ // 128
    qT = np.ascontiguousarray(q.transpose(0, 2, 1))  # [B, E, T]
    mrow = np.where(mask, np.float32(MASK_ROW), np.float32(0.0)).astype(np.float32)
    mcolv = np.where(mask, np.float32(MASK_COL), np.float32(0.0)).astype(np.float32)
    mcol = np.ascontiguousarray(
        mcolv.reshape(B, KCN, 128).transpose(2, 0, 1)
    )  # [128, B, KCN]

    in_maps = []
    for c in range(NCORES):
        hs = [HPC * c + i for i in range(HPC)]
        wq = np.concatenate([Wqkv[:, h * D : (h + 1) * D] for h in hs], axis=1)
        wk = np.concatenate(
            [Wqkv[:, E + h * D : E + (h + 1) * D] for h in hs], axis=1
        )
        wv = np.concatenate(
            [Wqkv[:, 2 * E + h * D : 2 * E + (h + 1) * D] for h in hs], axis=1
        )
        bq = np.concatenate([bqkv[h * D : (h + 1) * D] for h in hs]).reshape(128, 1)
        bk = np.concatenate(
            [bqkv[E + h * D : E + (h + 1) * D] for h in hs]
        ).reshape(128, 1)
        bv = np.concatenate([bqkv[2 * E + h * D : 2 * E + (h + 1) * D] for h in hs])
        wout = np.stack([Wout[h * D : (h + 1) * D, :] for h in hs], axis=1)
        in_maps.append(
            dict(
                qT=qT,
                wq=np.ascontiguousarray(wq),
                wk=np.ascontiguousarray(wk),
                wv=np.ascontiguousarray(wv),
                bq=np.ascontiguousarray(bq),
                bk=np.ascontiguousarray(bk),
                bv=np.ascontiguousarray(bv.reshape(128, 1)),
                mrow=mrow,
                mcol=mcol,
                wout=np.ascontiguousarray(wout),
            )
        )
    return in_maps


def assemble(results, bout):
    """Combine per-core results into (out, attn)."""
    bout = np.asarray(bout, dtype=np.float32)
    T_ = results[0]["outp"].shape[1]
    attn = np.empty((B, H, T_, T_), dtype=np.float32)
    out = np.zeros((B, T_, E), dtype=np.float32)
    for c, res in enumerate(results):
        a = res["attn"]  # [B, HPC, T, T]
        for i in range(HPC):
            attn[:, HPC * c + i] = a[:, i]
        out += res["outp"]
    out += bout
    return out, attn


_CACHE = {}


def _get_nc():
    if "nc" not in _CACHE:
        _CACHE["nc"] = build_nc(T)
    return _CACHE["nc"]


def kernel(q, k=None, v=None, mask=None, Wqkv=None, bqkv=None, Wout=None, bout=None):
    """Full-input entry point. k/v are unused (reference derives qkv from q)."""
    from concourse.bass_utils import run_bass_kernel_spmd

    nc = _get_nc()
    in_maps = make_in_maps(q, mask, Wqkv, bqkv, Wout, T_=T)
    res = run_bass_kernel_spmd(nc, in_maps, core_ids=list(range(NCORES)))
    return assemble(res.results, bout)


# revision 33
# speedup vs baseline: 1.5016x; 1.4670x over previous
"""Trainium2 Bass kernel: fused MHA (QKV proj -> masked softmax attention -> out proj).

Problem shapes: B=2, T=2048, E=1024, H=16, D=64.
Returns (out [B,T,E], attn [B,H,T,T]) matching the reference.

Sharding: heads across the 8 cores (2 heads/core, both batches on every core).

Key-compaction: masked keys contribute exactly 0 to softmax, attn and out, so
the host gathers the unmasked key positions (padded to a multiple of 128, KCAP)
and the kernel only computes over KCAP keys; the host scatters the attention
columns back into a zero tensor. This roughly halves the exp work, the score
matmuls, attn@V and the attention HBM write for ~50%-masked inputs.

Per core, for its 2 heads:
  - QKV projection in transposed layout (qhT [D,T] over all queries from qT;
    khT/vT [D,KCAP] over compacted keys from qTc)
  - scores twice on the tensor engine (fp32r full-rate):
      layout B [keys, q] -> exp -> U_B -> attnV matmuls (keys on partitions)
      layout A [q, keys] -> exp (+row-sum Z via accum_out) -> normalize -> attn
  - masking (only the padding lanes) folded into the matmuls / exp bias
  - per-head out-projection partials, normalized by 1/Z, summed on host.
"""

import numpy as np

B, T, E, H, D = 2, 2048, 1024, 16, 64
NCORES = 8
HPC = H // NCORES  # heads per core

MASK_ROW = -80000.0  # added pre-scale (x0.125 -> -1e4 -> exp == 0.0 in fp32)
MASK_COL = -10000.0  # added post-scale as activation bias


def _col_chunks(total, width):
    out = []
    o = 0
    while o < total:
        w = min(width, total - o)
        out.append((o, w))
        o += w
    return out


def build_nc(T_=T, KCAP=T):
    """Build + compile the Bass module (same program on all cores; per-core data
    differs only through the input tensors). KCAP = padded compacted key count."""
    from contextlib import ExitStack

    import concourse.tile as tile
    from concourse import bacc, mybir
    from concourse.masks import make_identity

    f32 = mybir.dt.float32
    f32r = mybir.dt.float32r
    Exp = mybir.ActivationFunctionType.Exp

    KS = E // 128          # contraction slices for projection
    TC = T_ // 128         # 128-row chunks of T (queries)
    KCN = KCAP // 128      # 128-wide key chunks
    CW = 1024 if T_ % 1024 == 0 else T_   # q-column group width (phase B)
    NCG = T_ // CW
    KGL = _col_chunks(KCAP, CW)           # key-column groups (phase A exps)
    NKG = len(KGL)

    nc = bacc.Bacc(
        "TRN2",
        target_bir_lowering=False,
        debug=False,
        enable_asserts=False,
        num_devices=NCORES,
    )

    qT_d = nc.dram_tensor("qT", (B, E, T_), f32r, kind="ExternalInput").ap()
    qTc_d = nc.dram_tensor("qTc", (B, E, KCAP), f32r, kind="ExternalInput").ap()
    wq_d = nc.dram_tensor("wq", (E, 128), f32r, kind="ExternalInput").ap()
    wk_d = nc.dram_tensor("wk", (E, 128), f32r, kind="ExternalInput").ap()
    wv_d = nc.dram_tensor("wv", (E, 128), f32r, kind="ExternalInput").ap()
    bq_d = nc.dram_tensor("bq", (128, 1), f32, kind="ExternalInput").ap()
    bk_d = nc.dram_tensor("bk", (128, 1), f32, kind="ExternalInput").ap()
    bv_d = nc.dram_tensor("bv", (128, 1), f32, kind="ExternalInput").ap()
    mrow_d = nc.dram_tensor("mrow", (B, KCAP), f32r, kind="ExternalInput").ap()
    mcol_d = nc.dram_tensor("mcol", (128, B, KCN), f32, kind="ExternalInput").ap()
    wout_d = nc.dram_tensor("wout", (64, 2, E), f32r, kind="ExternalInput").ap()
    attn_d = nc.dram_tensor(
        "attn", (B, HPC, T_, KCAP), f32, kind="ExternalOutput"
    ).ap()
    outp_d = nc.dram_tensor("outp", (B, T_, E), f32, kind="ExternalOutput").ap()

    def r(ap):  # operands already declared fp32r
        return ap

    with tile.TileContext(nc) as tc, ExitStack() as ctx:
        consts = ctx.enter_context(tc.tile_pool(name="consts", bufs=1))
        qpool = ctx.enter_context(tc.tile_pool(name="qts", bufs=4))
        sc = ctx.enter_context(tc.tile_pool(name="sc", bufs=2, space="PSUM"))
        pp = ctx.enter_context(tc.tile_pool(name="pp", bufs=2, space="PSUM"))
        po = ctx.enter_context(tc.tile_pool(name="po", bufs=1, space="PSUM"))
        ubp = ctx.enter_context(tc.tile_pool(name="ub", bufs=3))
        atp = ctx.enter_context(tc.tile_pool(name="at", bufs=4))
        hb = ctx.enter_context(tc.tile_pool(name="hb", bufs=1))   # per-b persistents
        zp_pool = ctx.enter_context(tc.tile_pool(name="zs", bufs=4))
        opool = ctx.enter_context(tc.tile_pool(name="op", bufs=4))
        tpool = ctx.enter_context(tc.tile_pool(name="tp", bufs=2))

        # ---- constants ----
        wq_sb = consts.tile([128, KS, 128], f32r, tag="wq")
        wk_sb = consts.tile([128, KS, 128], f32r, tag="wk")
        wv_sb = consts.tile([128, KS, 128], f32r, tag="wv")
        nc.sync.dma_start(wq_sb, wq_d.rearrange("(s p) m -> p s m", p=128))
        nc.sync.dma_start(wk_sb, wk_d.rearrange("(s p) m -> p s m", p=128))
        nc.sync.dma_start(wv_sb, wv_d.rearrange("(s p) m -> p s m", p=128))
        bq_sb = consts.tile([128, 1], f32, tag="bq")
        bk_sb = consts.tile([128, 1], f32, tag="bk")
        bv_sb = consts.tile([128, 1], f32, tag="bv")
        nc.sync.dma_start(bq_sb, bq_d)
        nc.sync.dma_start(bk_sb, bk_d)
        nc.sync.dma_start(bv_sb, bv_d)
        mcol_sb = consts.tile([128, B, KCN], f32, tag="mcol")
        nc.sync.dma_start(mcol_sb, mcol_d)
        wout_sb = consts.tile([64, 2, E], f32r, tag="wout")
        nc.sync.dma_start(wout_sb, wout_d)
        ident0 = consts.tile([128, 128], f32, tag="ident0")
        make_identity(nc, ident0)
        ident = consts.tile([128, 128], f32r, tag="ident")
        nc.vector.tensor_copy(ident, ident0)
        ones0 = consts.tile([1, T_], f32, tag="ones0")
        nc.vector.memset(ones0, 1.0)

        for b in range(B):
            # ---- per-b persistent tiles ----
            qA = [hb.tile([65, T_], f32r, tag=f"qA{hh}", name=f"qA{hh}") for hh in range(2)]
            kA = [hb.tile([65, KCAP], f32r, tag=f"kA{hh}", name=f"kA{hh}") for hh in range(2)]
            vT_sb = hb.tile([128, KCAP], f32r, tag="vT")
            vh = hb.tile([128, KCN, 128], f32r, tag="vh")
            outT = [hb.tile([64, T_], f32r, tag=f"oT{hh}", name=f"oT{hh}") for hh in range(2)]
            for hh in range(2):
                nc.vector.tensor_copy(qA[hh][64:65, :], ones0)
                nc.sync.dma_start(kA[hh][64:65, :], mrow_d[b : b + 1, :])

            # ---- projection (pass-sequential, one psum slot at a time) ----
            # Q side: all T query positions from qT.
            for tco, tcw in _col_chunks(T_, 512):
                tsl = slice(tco, tco + tcw)
                qts_l = []
                for ks in range(KS):
                    qts = qpool.tile(
                        [128, 512], f32r, tag="qt", name="qts", bufs=16
                    )
                    nc.sync.dma_start(
                        qts[:, :tcw], qT_d[b, ks * 128 : (ks + 1) * 128, tsl]
                    )
                    qts_l.append(qts)
                ps = pp.tile([128, 512], f32, tag="pp", name="psprq")
                for ks in range(KS):
                    nc.tensor.matmul(
                        ps[:, :tcw],
                        lhsT=r(wq_sb[:, ks, :]),
                        rhs=r(qts_l[ks][:, :tcw]),
                        start=(ks == 0),
                        stop=(ks == KS - 1),
                        skip_group_check=True,
                    )
                nc.vector.tensor_scalar_add(
                    qA[0][0:64, tsl], ps[0:64, :tcw], bq_sb[0:64, 0:1]
                )
                shtmp = tpool.tile([128, 512], f32r, tag="sh", name="shtmp")
                nc.vector.tensor_scalar_add(
                    shtmp[64:128, :tcw], ps[64:128, :tcw], bq_sb[64:128, 0:1]
                )
                nc.sync.dma_start(qA[1][0:64, tsl], shtmp[64:128, :tcw])

            # K/V side: compacted key positions from qTc.
            for kco, kcw in _col_chunks(KCAP, 512):
                ksl = slice(kco, kco + kcw)
                qts_l = []
                for ks in range(KS):
                    qts = qpool.tile(
                        [128, 512], f32r, tag="qt", name="qts", bufs=16
                    )
                    nc.sync.dma_start(
                        qts[:, :kcw], qTc_d[b, ks * 128 : (ks + 1) * 128, ksl]
                    )
                    qts_l.append(qts)
                for wsb, dst, bias in ((wk_sb, kA, bk_sb), (wv_sb, None, bv_sb)):
                    ps = pp.tile([128, 512], f32, tag="pp", name="psprk")
                    for ks in range(KS):
                        nc.tensor.matmul(
                            ps[:, :kcw],
                            lhsT=r(wsb[:, ks, :]),
                            rhs=r(qts_l[ks][:, :kcw]),
                            start=(ks == 0),
                            stop=(ks == KS - 1),
                            skip_group_check=True,
                        )
                    if dst is None:
                        nc.vector.tensor_scalar_add(
                            vT_sb[:, ksl], ps[:, :kcw], bias[:, 0:1]
                        )
                    else:
                        nc.vector.tensor_scalar_add(
                            dst[0][0:64, ksl], ps[0:64, :kcw], bias[0:64, 0:1]
                        )
                        shtmp = tpool.tile(
                            [128, 512], f32r, tag="sh", name="shtmp"
                        )
                        nc.vector.tensor_scalar_add(
                            shtmp[64:128, :kcw], ps[64:128, :kcw], bias[64:128, 0:1]
                        )
                        nc.sync.dma_start(dst[1][0:64, ksl], shtmp[64:128, :kcw])

            # ---- vh = vT.T per 128-chunk (PE transpose) ----
            for kc in range(KCN):
                pst = pp.tile([128, 128], f32r, tag="pp", name="pst")
                nc.tensor.transpose(pst, vT_sb[:, kc * 128 : (kc + 1) * 128], ident)
                nc.vector.tensor_copy(vh[:, kc, :], pst)

            rz_h = [None, None]

            def phaseAB(hh):
                """Interleaved emission of phase A (attn rows: DMA-heavy) and
                phase B (U_B @ V: ACT/PE-heavy) so ACT and DMA co-saturate."""
                zps = zp_pool.tile(
                    [128, TC, NKG], f32, tag=f"zp{hh}", name=f"zp{hh}", bufs=2
                )
                rz = zp_pool.tile(
                    [128, TC], f32, tag=f"rz{hh}", name=f"rz{hh}", bufs=2
                )
                rz_h[hh] = rz
                otp_l = [None] * NCG

                def b_unit(j):
                    half, kc = divmod(j, KCN)
                    if kc == 0:
                        otp_l[half] = po.tile([64, CW], f32, tag="po", name="otp")
                    otp = otp_l[half]
                    scb = sc.tile([128, CW], f32, tag="sc", name="scb")
                    for o, w in _col_chunks(CW, 512):
                        nc.tensor.matmul(
                            scb[:, o : o + w],
                            lhsT=r(kA[hh][0:64, kc * 128 : (kc + 1) * 128]),
                            rhs=r(qA[hh][0:64, half * CW + o : half * CW + o + w]),
                            start=True,
                            stop=True,
                        )
                    ubt = ubp.tile([128, CW], f32r, tag="ub", name="ubt")
                    nc.scalar.activation(
                        ubt, scb, Exp, bias=mcol_sb[:, b, kc : kc + 1], scale=0.125
                    )
                    for o, w in _col_chunks(CW, 512):
                        nc.tensor.matmul(
                            otp[:, o : o + w],
                            lhsT=r(vh[:, kc, hh * 64 : (hh + 1) * 64]),
                            rhs=r(ubt[:, o : o + w]),
                            start=(kc == 0),
                            stop=(kc == KCN - 1),
                            skip_group_check=True,
                        )
                    if kc == KCN - 1:
                        nc.vector.tensor_copy(
                            outT[hh][:, half * CW : (half + 1) * CW], otp
                        )

                def a_unit(tci):
                    tsl = slice(tci * 128, (tci + 1) * 128)
                    att = atp.tile([128, KCAP], f32, tag="at", name="att")
                    for cg, (ko, kw) in enumerate(KGL):
                        sca = sc.tile([128, CW], f32, tag="sc", name="sca")
                        for o, w in _col_chunks(kw, 512):
                            nc.tensor.matmul(
                                sca[:, o : o + w],
                                lhsT=r(qA[hh][:, tsl]),
                                rhs=r(kA[hh][:, ko + o : ko + o + w]),
                                start=True,
                                stop=True,
                            )
                        nc.scalar.activation(
                            att[:, ko : ko + kw],
                            sca[:, :kw],
                            Exp,
                            scale=0.125,
                            accum_out=zps[:, tci, cg : cg + 1],
                        )
                    ztmp = zp_pool.tile([128, 1], f32, tag="ztmp", name="ztmp", bufs=3)
                    nc.vector.tensor_reduce(
                        ztmp,
                        zps[:, tci, :],
                        axis=mybir.AxisListType.X,
                        op=mybir.AluOpType.add,
                    )
                    nc.vector.reciprocal(rz[:, tci : tci + 1], ztmp)
                    nc.vector.tensor_scalar_mul(att, att, rz[:, tci : tci + 1])
                    nc.sync.dma_start(attn_d[b, hh, tsl, :], att)

                nb = NCG * KCN
                jprev = 0
                for tci in range(TC):
                    a_unit(tci)
                    jnext = (tci + 1) * nb // TC
                    for j in range(jprev, jnext):
                        b_unit(j)
                    jprev = jnext

            def outproj():
                for tci in range(TC):
                    tsl = slice(tci * 128, (tci + 1) * 128)
                    pss = []
                    for hh in range(2):
                        ps = sc.tile([128, E], f32, tag="sc", name="psop")
                        for eo, ew in _col_chunks(E, 512):
                            nc.tensor.matmul(
                                ps[:, eo : eo + ew],
                                lhsT=r(outT[hh][:, tsl]),
                                rhs=r(wout_sb[:, hh, eo : eo + ew]),
                                start=True,
                                stop=True,
                            )
                        pss.append(ps)
                    t0 = tpool.tile([128, E], f32, tag="tp", name="t0")
                    nc.scalar.mul(t0, pss[0], rz_h[0][:, tci : tci + 1])
                    ot = opool.tile([128, E], f32, tag="op", name="ot")
                    nc.vector.tensor_scalar_mul(
                        ot, pss[1], rz_h[1][:, tci : tci + 1]
                    )
                    nc.vector.tensor_add(ot, ot, t0)
                    nc.sync.dma_start(outp_d[b, tsl, :], ot)

            phaseAB(0)
            phaseAB(1)
            outproj()

    nc.compile()
    return nc


def _key_compaction(mask, T_):
    """Per-batch unmasked key indices, padded to a common multiple of 128."""
    idxs = [np.where(~mask[b])[0] for b in range(B)]
    kmax = max(1, max(len(i) for i in idxs))
    KCAP = min(T_, ((kmax + 127) // 128) * 128)
    pidxs = []
    for i in idxs:
        pad = np.zeros(KCAP - len(i), dtype=i.dtype)  # padded lanes masked out
        pidxs.append(np.concatenate([i, pad]))
    return idxs, pidxs, KCAP


def make_in_maps(q, mask, Wqkv, bqkv, Wout, T_=T):
    """Per-core input dicts (host-side sharding by heads + key compaction)."""
    q = np.ascontiguousarray(np.asarray(q, dtype=np.float32))
    mask = np.asarray(mask).astype(bool)
    Wqkv = np.asarray(Wqkv, dtype=np.float32)
    bqkv = np.asarray(bqkv, dtype=np.float32)
    Wout = np.asarray(Wout, dtype=np.float32)

    idxs, pidxs, KCAP = _key_compaction(mask, T_)
    KCN = KCAP // 128
    qT = np.ascontiguousarray(q.transpose(0, 2, 1))  # [B, E, T]
    qTc = np.stack([qT[b][:, pidxs[b]] for b in range(B)])  # [B, E, KCAP]
    # padding lanes (j >= len(idxs[b])) are masked; real lanes are unmasked
    mrow = np.zeros((B, KCAP), np.float32)
    mcolv = np.zeros((B, KCAP), np.float32)
    for b in range(B):
        mrow[b, len(idxs[b]) :] = MASK_ROW
        mcolv[b, len(idxs[b]) :] = MASK_COL
    mcol = np.ascontiguousarray(mcolv.reshape(B, KCN, 128).transpose(2, 0, 1))

    in_maps = []
    for c in range(NCORES):
        hs = [HPC * c + i for i in range(HPC)]
        wq = np.concatenate([Wqkv[:, h * D : (h + 1) * D] for h in hs], axis=1)
        wk = np.concatenate(
            [Wqkv[:, E + h * D : E + (h + 1) * D] for h in hs], axis=1
        )
        wv = np.concatenate(
            [Wqkv[:, 2 * E + h * D : 2 * E + (h + 1) * D] for h in hs], axis=1
        )
        bq = np.concatenate([bqkv[h * D : (h + 1) * D] for h in hs]).reshape(128, 1)
        bk = np.concatenate(
            [bqkv[E + h * D : E + (h + 1) * D] for h in hs]
        ).reshape(128, 1)
        bv = np.concatenate([bqkv[2 * E + h * D : 2 * E + (h + 1) * D] for h in hs])
        wout = np.stack([Wout[h * D : (h + 1) * D, :] for h in hs], axis=1)
        in_maps.append(
            dict(
                qT=qT,
                qTc=np.ascontiguousarray(qTc),
                wq=np.ascontiguousarray(wq),
                wk=np.ascontiguousarray(wk),
                wv=np.ascontiguousarray(wv),
                bq=np.ascontiguousarray(bq),
                bk=np.ascontiguousarray(bk),
                bv=np.ascontiguousarray(bv.reshape(128, 1)),
                mrow=mrow,
                mcol=mcol,
                wout=np.ascontiguousarray(wout),
            )
        )
    return in_maps, idxs, KCAP


def assemble(results, bout, idxs, T_=T):
    """Combine per-core results into (out, attn), scattering compacted keys."""
    bout = np.asarray(bout, dtype=np.float32)
    attn = np.zeros((B, H, T_, T_), dtype=np.float32)
    out = np.zeros((B, T_, E), dtype=np.float32)
    for c, res in enumerate(results):
        a = res["attn"]  # [B, HPC, T, KCAP]
        for bb in range(B):
            n = len(idxs[bb])
            for i in range(HPC):
                attn[bb, HPC * c + i][:, idxs[bb]] = a[bb, i][:, :n]
        out += res["outp"]
    out += bout
    return out, attn


_CACHE = {}


def _get_nc(KCAP=T):
    key = ("nc", KCAP)
    if key not in _CACHE:
        _CACHE[key] = build_nc(T, KCAP)
    return _CACHE[key]


def kernel(q, k=None, v=None, mask=None, Wqkv=None, bqkv=None, Wout=None, bout=None):
    """Full-input entry point. k/v are unused (reference derives qkv from q)."""
    from concourse.bass_utils import run_bass_kernel_spmd

    in_maps, idxs, KCAP = make_in_maps(q, mask, Wqkv, bqkv, Wout, T_=T)
    nc = _get_nc(KCAP)
    res = run_bass_kernel_spmd(nc, in_maps, core_ids=list(range(NCORES)))
    return assemble(res.results, bout, idxs, T_=T)


# revision 34
# speedup vs baseline: 1.5559x; 1.0362x over previous
"""Trainium2 Bass kernel: fused MHA (QKV proj -> masked softmax attention -> out proj).

Problem shapes: B=2, T=2048, E=1024, H=16, D=64.
Returns (out [B,T,E], attn [B,H,T,T]) matching the reference.

Sharding: heads across the 8 cores (2 heads/core, both batches on every core).

Key-compaction: masked keys contribute exactly 0 to softmax, attn and out, so
the host gathers the unmasked key positions (padded to a multiple of 128, KCAP)
and the kernel only computes over KCAP keys; the host scatters the attention
columns back into a zero tensor. This roughly halves the exp work, the score
matmuls, attn@V and the attention HBM write for ~50%-masked inputs.

Per core, for its 2 heads:
  - QKV projection in transposed layout (qhT [D,T] over all queries from qT;
    khT/vT [D,KCAP] over compacted keys from qTc)
  - scores twice on the tensor engine (fp32r full-rate):
      layout B [keys, q] -> exp -> U_B -> attnV matmuls (keys on partitions)
      layout A [q, keys] -> exp (+row-sum Z via accum_out) -> normalize -> attn
  - masking (only the padding lanes) folded into the matmuls / exp bias
  - per-head out-projection partials, normalized by 1/Z, summed on host.
"""

import numpy as np

B, T, E, H, D = 2, 2048, 1024, 16, 64
NCORES = 8
HPC = H // NCORES  # heads per core

MASK_ROW = -80000.0  # added pre-scale (x0.125 -> -1e4 -> exp == 0.0 in fp32)
MASK_COL = -10000.0  # added post-scale as activation bias


def _col_chunks(total, width):
    out = []
    o = 0
    while o < total:
        w = min(width, total - o)
        out.append((o, w))
        o += w
    return out


def build_nc(T_=T, KCAP=T):
    """Build + compile the Bass module (same program on all cores; per-core data
    differs only through the input tensors). KCAP = padded compacted key count."""
    from contextlib import ExitStack

    import concourse.tile as tile
    from concourse import bacc, mybir
    from concourse.masks import make_identity

    f32 = mybir.dt.float32
    f32r = mybir.dt.float32r
    Exp = mybir.ActivationFunctionType.Exp

    KS = E // 128          # contraction slices for projection
    TC = T_ // 128         # 128-row chunks of T (queries)
    KCN = KCAP // 128      # 128-wide key chunks
    CW = 1024 if T_ % 1024 == 0 else T_   # q-column group width (phase B)
    NCG = T_ // CW
    KGL = _col_chunks(KCAP, CW)           # key-column groups (phase A exps)
    NKG = len(KGL)

    nc = bacc.Bacc(
        "TRN2",
        target_bir_lowering=False,
        debug=False,
        enable_asserts=False,
        num_devices=NCORES,
    )

    qT_d = nc.dram_tensor("qT", (B, E, T_), f32r, kind="ExternalInput").ap()
    qTc_d = nc.dram_tensor("qTc", (B, E, KCAP), f32r, kind="ExternalInput").ap()
    wq_d = nc.dram_tensor("wq", (E, 128), f32r, kind="ExternalInput").ap()
    wk_d = nc.dram_tensor("wk", (E, 128), f32r, kind="ExternalInput").ap()
    wv_d = nc.dram_tensor("wv", (E, 128), f32r, kind="ExternalInput").ap()
    bq_d = nc.dram_tensor("bq", (128, 1), f32, kind="ExternalInput").ap()
    bk_d = nc.dram_tensor("bk", (128, 1), f32, kind="ExternalInput").ap()
    bv_d = nc.dram_tensor("bv", (128, 1), f32, kind="ExternalInput").ap()
    mrow_d = nc.dram_tensor("mrow", (B, KCAP), f32r, kind="ExternalInput").ap()
    mcol_d = nc.dram_tensor("mcol", (128, B, KCN), f32, kind="ExternalInput").ap()
    wout_d = nc.dram_tensor("wout", (64, 2, E), f32r, kind="ExternalInput").ap()
    attn_d = nc.dram_tensor(
        "attn", (B, HPC, T_, KCAP), f32, kind="ExternalOutput"
    ).ap()
    outp_d = nc.dram_tensor("outp", (B, T_, E), f32, kind="ExternalOutput").ap()

    def r(ap):  # operands already declared fp32r
        return ap

    with tile.TileContext(nc) as tc, ExitStack() as ctx:
        consts = ctx.enter_context(tc.tile_pool(name="consts", bufs=1))
        qpool = ctx.enter_context(tc.tile_pool(name="qts", bufs=4))
        sc = ctx.enter_context(tc.tile_pool(name="sc", bufs=2, space="PSUM"))
        pp = ctx.enter_context(tc.tile_pool(name="pp", bufs=2, space="PSUM"))
        po = ctx.enter_context(tc.tile_pool(name="po", bufs=1, space="PSUM"))
        ubp = ctx.enter_context(tc.tile_pool(name="ub", bufs=3))
        atp = ctx.enter_context(tc.tile_pool(name="at", bufs=4))
        hb = ctx.enter_context(tc.tile_pool(name="hb", bufs=1))   # per-b persistents
        zp_pool = ctx.enter_context(tc.tile_pool(name="zs", bufs=4))
        opool = ctx.enter_context(tc.tile_pool(name="op", bufs=4))
        tpool = ctx.enter_context(tc.tile_pool(name="tp", bufs=2))

        # ---- constants ----
        wq_sb = consts.tile([128, KS, 128], f32r, tag="wq")
        wk_sb = consts.tile([128, KS, 128], f32r, tag="wk")
        wv_sb = consts.tile([128, KS, 128], f32r, tag="wv")
        nc.sync.dma_start(wq_sb, wq_d.rearrange("(s p) m -> p s m", p=128))
        nc.sync.dma_start(wk_sb, wk_d.rearrange("(s p) m -> p s m", p=128))
        nc.sync.dma_start(wv_sb, wv_d.rearrange("(s p) m -> p s m", p=128))
        bq_sb = consts.tile([128, 1], f32, tag="bq")
        bk_sb = consts.tile([128, 1], f32, tag="bk")
        bv_sb = consts.tile([128, 1], f32, tag="bv")
        nc.sync.dma_start(bq_sb, bq_d)
        nc.sync.dma_start(bk_sb, bk_d)
        nc.sync.dma_start(bv_sb, bv_d)
        mcol_sb = consts.tile([128, B, KCN], f32, tag="mcol")
        nc.sync.dma_start(mcol_sb, mcol_d)
        wout_sb = consts.tile([64, 2, E], f32r, tag="wout")
        nc.sync.dma_start(wout_sb, wout_d)
        ident0 = consts.tile([128, 128], f32, tag="ident0")
        make_identity(nc, ident0)
        ident = consts.tile([128, 128], f32r, tag="ident")
        nc.vector.tensor_copy(ident, ident0)
        ones0 = consts.tile([1, T_], f32, tag="ones0")
        nc.vector.memset(ones0, 1.0)

        for b in range(B):
            # ---- per-b persistent tiles ----
            qA = [hb.tile([65, T_], f32r, tag=f"qA{hh}", name=f"qA{hh}") for hh in range(2)]
            kA = [hb.tile([65, KCAP], f32r, tag=f"kA{hh}", name=f"kA{hh}") for hh in range(2)]
            vT_sb = hb.tile([128, KCAP], f32r, tag="vT")
            vh = hb.tile([128, KCN, 128], f32r, tag="vh")
            outT = [hb.tile([64, T_], f32r, tag=f"oT{hh}", name=f"oT{hh}") for hh in range(2)]
            for hh in range(2):
                nc.vector.tensor_copy(qA[hh][64:65, :], ones0)
                nc.sync.dma_start(kA[hh][64:65, :], mrow_d[b : b + 1, :])

            # ---- projection (pass-sequential, one psum slot at a time) ----
            # Q side: all T query positions from qT.
            for tco, tcw in _col_chunks(T_, 512):
                tsl = slice(tco, tco + tcw)
                qts_l = []
                for ks in range(KS):
                    qts = qpool.tile(
                        [128, 512], f32r, tag="qt", name="qts", bufs=16
                    )
                    nc.sync.dma_start(
                        qts[:, :tcw], qT_d[b, ks * 128 : (ks + 1) * 128, tsl]
                    )
                    qts_l.append(qts)
                ps = pp.tile([128, 512], f32, tag="pp", name="psprq")
                for ks in range(KS):
                    nc.tensor.matmul(
                        ps[:, :tcw],
                        lhsT=r(wq_sb[:, ks, :]),
                        rhs=r(qts_l[ks][:, :tcw]),
                        start=(ks == 0),
                        stop=(ks == KS - 1),
                        skip_group_check=True,
                    )
                nc.vector.tensor_scalar_add(
                    qA[0][0:64, tsl], ps[0:64, :tcw], bq_sb[0:64, 0:1]
                )
                shtmp = tpool.tile([128, 512], f32r, tag="sh", name="shtmp")
                nc.vector.tensor_scalar_add(
                    shtmp[64:128, :tcw], ps[64:128, :tcw], bq_sb[64:128, 0:1]
                )
                nc.sync.dma_start(qA[1][0:64, tsl], shtmp[64:128, :tcw])

            # K/V side: compacted key positions from qTc.
            for kco, kcw in _col_chunks(KCAP, 512):
                ksl = slice(kco, kco + kcw)
                qts_l = []
                for ks in range(KS):
                    qts = qpool.tile(
                        [128, 512], f32r, tag="qt", name="qts", bufs=16
                    )
                    nc.sync.dma_start(
                        qts[:, :kcw], qTc_d[b, ks * 128 : (ks + 1) * 128, ksl]
                    )
                    qts_l.append(qts)
                for wsb, dst, bias in ((wk_sb, kA, bk_sb), (wv_sb, None, bv_sb)):
                    ps = pp.tile([128, 512], f32, tag="pp", name="psprk")
                    for ks in range(KS):
                        nc.tensor.matmul(
                            ps[:, :kcw],
                            lhsT=r(wsb[:, ks, :]),
                            rhs=r(qts_l[ks][:, :kcw]),
                            start=(ks == 0),
                            stop=(ks == KS - 1),
                            skip_group_check=True,
                        )
                    if dst is None:
                        nc.vector.tensor_scalar_add(
                            vT_sb[:, ksl], ps[:, :kcw], bias[:, 0:1]
                        )
                    else:
                        nc.vector.tensor_scalar_add(
                            dst[0][0:64, ksl], ps[0:64, :kcw], bias[0:64, 0:1]
                        )
                        shtmp = tpool.tile(
                            [128, 512], f32r, tag="sh", name="shtmp"
                        )
                        nc.vector.tensor_scalar_add(
                            shtmp[64:128, :kcw], ps[64:128, :kcw], bias[64:128, 0:1]
                        )
                        nc.sync.dma_start(dst[1][0:64, ksl], shtmp[64:128, :kcw])

            # ---- vh = vT.T per 128-chunk (PE transpose) ----
            for kc in range(KCN):
                pst = pp.tile([128, 128], f32r, tag="pp", name="pst")
                nc.tensor.transpose(pst, vT_sb[:, kc * 128 : (kc + 1) * 128], ident)
                nc.vector.tensor_copy(vh[:, kc, :], pst)

            rz_h = [None, None]

            def phaseAB(hh):
                """Interleaved emission of phase A (attn rows: DMA-heavy) and
                phase B (U_B @ V: ACT/PE-heavy) so ACT and DMA co-saturate."""
                zps = zp_pool.tile(
                    [128, TC, NKG], f32, tag=f"zp{hh}", name=f"zp{hh}", bufs=2
                )
                rz = zp_pool.tile(
                    [128, TC], f32, tag=f"rz{hh}", name=f"rz{hh}", bufs=2
                )
                rz_h[hh] = rz
                otp_l = [None] * NCG

                def b_unit(j):
                    half, kc = divmod(j, KCN)
                    if kc == 0:
                        otp_l[half] = po.tile([64, CW], f32, tag="po", name="otp")
                    otp = otp_l[half]
                    scb = sc.tile([128, CW], f32, tag="sc", name="scb")
                    for o, w in _col_chunks(CW, 512):
                        nc.tensor.matmul(
                            scb[:, o : o + w],
                            lhsT=r(kA[hh][0:64, kc * 128 : (kc + 1) * 128]),
                            rhs=r(qA[hh][0:64, half * CW + o : half * CW + o + w]),
                            start=True,
                            stop=True,
                        )
                    ubt = ubp.tile([128, CW], f32r, tag="ub", name="ubt")
                    nc.scalar.activation(
                        ubt, scb, Exp, bias=mcol_sb[:, b, kc : kc + 1], scale=0.125
                    )
                    for o, w in _col_chunks(CW, 512):
                        nc.tensor.matmul(
                            otp[:, o : o + w],
                            lhsT=r(vh[:, kc, hh * 64 : (hh + 1) * 64]),
                            rhs=r(ubt[:, o : o + w]),
                            start=(kc == 0),
                            stop=(kc == KCN - 1),
                            skip_group_check=True,
                        )
                    if kc == KCN - 1:
                        nc.vector.tensor_copy(
                            outT[hh][:, half * CW : (half + 1) * CW], otp
                        )

                def a_unit(tci):
                    tsl = slice(tci * 128, (tci + 1) * 128)
                    att = atp.tile([128, KCAP], f32, tag="at", name="att")
                    for cg, (ko, kw) in enumerate(KGL):
                        sca = sc.tile([128, CW], f32, tag="sc", name="sca")
                        for o, w in _col_chunks(kw, 512):
                            nc.tensor.matmul(
                                sca[:, o : o + w],
                                lhsT=r(qA[hh][:, tsl]),
                                rhs=r(kA[hh][:, ko + o : ko + o + w]),
                                start=True,
                                stop=True,
                            )
                        nc.scalar.activation(
                            att[:, ko : ko + kw],
                            sca[:, :kw],
                            Exp,
                            scale=0.125,
                            accum_out=zps[:, tci, cg : cg + 1],
                        )
                    ztmp = zp_pool.tile([128, 1], f32, tag="ztmp", name="ztmp", bufs=3)
                    nc.vector.tensor_reduce(
                        ztmp,
                        zps[:, tci, :],
                        axis=mybir.AxisListType.X,
                        op=mybir.AluOpType.add,
                    )
                    nc.vector.reciprocal(rz[:, tci : tci + 1], ztmp)
                    nc.vector.tensor_scalar_mul(att, att, rz[:, tci : tci + 1])
                    nc.sync.dma_start(attn_d[b, hh, tsl, :], att)

                nb = NCG * KCN
                jprev = 0
                for tci in range(TC):
                    a_unit(tci)
                    jnext = (tci + 1) * nb // TC
                    for j in range(jprev, jnext):
                        b_unit(j)
                    jprev = jnext

            def outproj():
                for tci in range(TC):
                    tsl = slice(tci * 128, (tci + 1) * 128)
                    pss = []
                    for hh in range(2):
                        ps = sc.tile([128, E], f32, tag="sc", name="psop")
                        for eo, ew in _col_chunks(E, 512):
                            nc.tensor.matmul(
                                ps[:, eo : eo + ew],
                                lhsT=r(outT[hh][:, tsl]),
                                rhs=r(wout_sb[:, hh, eo : eo + ew]),
                                start=True,
                                stop=True,
                            )
                        pss.append(ps)
                    t0 = tpool.tile([128, E], f32, tag="tp", name="t0")
                    nc.scalar.mul(t0, pss[0], rz_h[0][:, tci : tci + 1])
                    ot = opool.tile([128, E], f32, tag="op", name="ot")
                    nc.scalar.mul(ot, pss[1], rz_h[1][:, tci : tci + 1])
                    nc.vector.tensor_add(ot, ot, t0)
                    nc.sync.dma_start(outp_d[b, tsl, :], ot)

            phaseAB(0)
            phaseAB(1)
            outproj()

    nc.compile()
    return nc


def _key_compaction(mask, T_):
    """Per-batch unmasked key indices, padded to a common multiple of 128."""
    idxs = [np.where(~mask[b])[0] for b in range(B)]
    kmax = max(1, max(len(i) for i in idxs))
    KCAP = min(T_, ((kmax + 127) // 128) * 128)
    pidxs = []
    for i in idxs:
        pad = np.zeros(KCAP - len(i), dtype=i.dtype)  # padded lanes masked out
        pidxs.append(np.concatenate([i, pad]))
    return idxs, pidxs, KCAP


def make_in_maps(q, mask, Wqkv, bqkv, Wout, T_=T):
    """Per-core input dicts (host-side sharding by heads + key compaction)."""
    q = np.ascontiguousarray(np.asarray(q, dtype=np.float32))
    mask = np.asarray(mask).astype(bool)
    Wqkv = np.asarray(Wqkv, dtype=np.float32)
    bqkv = np.asarray(bqkv, dtype=np.float32)
    Wout = np.asarray(Wout, dtype=np.float32)

    idxs, pidxs, KCAP = _key_compaction(mask, T_)
    KCN = KCAP // 128
    qT = np.ascontiguousarray(q.transpose(0, 2, 1))  # [B, E, T]
    qTc = np.stack([qT[b][:, pidxs[b]] for b in range(B)])  # [B, E, KCAP]
    # padding lanes (j >= len(idxs[b])) are masked; real lanes are unmasked
    mrow = np.zeros((B, KCAP), np.float32)
    mcolv = np.zeros((B, KCAP), np.float32)
    for b in range(B):
        mrow[b, len(idxs[b]) :] = MASK_ROW
        mcolv[b, len(idxs[b]) :] = MASK_COL
    mcol = np.ascontiguousarray(mcolv.reshape(B, KCN, 128).transpose(2, 0, 1))

    in_maps = []
    for c in range(NCORES):
        hs = [HPC * c + i for i in range(HPC)]
        wq = np.concatenate([Wqkv[:, h * D : (h + 1) * D] for h in hs], axis=1)
        wk = np.concatenate(
            [Wqkv[:, E + h * D : E + (h + 1) * D] for h in hs], axis=1
        )
        wv = np.concatenate(
            [Wqkv[:, 2 * E + h * D : 2 * E + (h + 1) * D] for h in hs], axis=1
        )
        bq = np.concatenate([bqkv[h * D : (h + 1) * D] for h in hs]).reshape(128, 1)
        bk = np.concatenate(
            [bqkv[E + h * D : E + (h + 1) * D] for h in hs]
        ).reshape(128, 1)
        bv = np.concatenate([bqkv[2 * E + h * D : 2 * E + (h + 1) * D] for h in hs])
        wout = np.stack([Wout[h * D : (h + 1) * D, :] for h in hs], axis=1)
        in_maps.append(
            dict(
                qT=qT,
                qTc=np.ascontiguousarray(qTc),
                wq=np.ascontiguousarray(wq),
                wk=np.ascontiguousarray(wk),
                wv=np.ascontiguousarray(wv),
                bq=np.ascontiguousarray(bq),
                bk=np.ascontiguousarray(bk),
                bv=np.ascontiguousarray(bv.reshape(128, 1)),
                mrow=mrow,
                mcol=mcol,
                wout=np.ascontiguousarray(wout),
            )
        )
    return in_maps, idxs, KCAP


def assemble(results, bout, idxs, T_=T):
    """Combine per-core results into (out, attn), scattering compacted keys."""
    bout = np.asarray(bout, dtype=np.float32)
    attn = np.zeros((B, H, T_, T_), dtype=np.float32)
    out = np.zeros((B, T_, E), dtype=np.float32)
    for c, res in enumerate(results):
        a = res["attn"]  # [B, HPC, T, KCAP]
        for bb in range(B):
            n = len(idxs[bb])
            for i in range(HPC):
                attn[bb, HPC * c + i][:, idxs[bb]] = a[bb, i][:, :n]
        out += res["outp"]
    out += bout
    return out, attn


_CACHE = {}


def _get_nc(KCAP=T):
    key = ("nc", KCAP)
    if key not in _CACHE:
        _CACHE[key] = build_nc(T, KCAP)
    return _CACHE[key]


def kernel(q, k=None, v=None, mask=None, Wqkv=None, bqkv=None, Wout=None, bout=None):
    """Full-input entry point. k/v are unused (reference derives qkv from q)."""
    from concourse.bass_utils import run_bass_kernel_spmd

    in_maps, idxs, KCAP = make_in_maps(q, mask, Wqkv, bqkv, Wout, T_=T)
    nc = _get_nc(KCAP)
    res = run_bass_kernel_spmd(nc, in_maps, core_ids=list(range(NCORES)))
    return assemble(res.results, bout, idxs, T_=T)


# revision 41
# speedup vs baseline: 1.5960x; 1.0257x over previous
"""Trainium2 Bass kernel: fused MHA (QKV proj -> masked softmax attention -> out proj).

Problem shapes: B=2, T=2048, E=1024, H=16, D=64.
Returns (out [B,T,E], attn [B,H,T,T]) matching the reference.

Sharding: heads across the 8 cores (2 heads/core, both batches on every core).

Key-compaction: masked keys contribute exactly 0 to softmax, attn and out, so
the host gathers the unmasked key positions (padded to a multiple of 128, KCAP)
and the kernel only computes over KCAP keys; the host scatters the attention
columns back into a zero tensor. This roughly halves the exp work, the score
matmuls, attn@V and the attention HBM write for ~50%-masked inputs.

Per core, for its 2 heads:
  - QKV projection in transposed layout (qhT [D,T] over all queries from qT;
    khT/vT [D,KCAP] over compacted keys from qTc)
  - scores twice on the tensor engine (fp32r full-rate):
      layout B [keys, q] -> exp -> U_B -> attnV matmuls (keys on partitions)
      layout A [q, keys] -> exp (+row-sum Z via accum_out) -> normalize -> attn
  - masking (only the padding lanes) folded into the matmuls / exp bias
  - per-head out-projection partials, normalized by 1/Z, summed on host.
"""

import numpy as np

B, T, E, H, D = 2, 2048, 1024, 16, 64
NCORES = 8
HPC = H // NCORES  # heads per core

MASK_ROW = -80000.0  # added pre-scale (x0.125 -> -1e4 -> exp == 0.0 in fp32)
MASK_COL = -10000.0  # added post-scale as activation bias


def _col_chunks(total, width):
    out = []
    o = 0
    while o < total:
        w = min(width, total - o)
        out.append((o, w))
        o += w
    return out


def build_nc(T_=T, KCAP=T):
    """Build + compile the Bass module (same program on all cores; per-core data
    differs only through the input tensors). KCAP = padded compacted key count."""
    from contextlib import ExitStack

    import concourse.tile as tile
    from concourse import bacc, mybir
    from concourse.masks import make_identity

    f32 = mybir.dt.float32
    f32r = mybir.dt.float32r
    Exp = mybir.ActivationFunctionType.Exp

    KS = E // 128          # contraction slices for projection
    TC = T_ // 128         # 128-row chunks of T (queries)
    KCN = KCAP // 128      # 128-wide key chunks
    CW = 1024 if T_ % 1024 == 0 else T_   # q-column group width (phase B)
    NCG = T_ // CW
    KGL = _col_chunks(KCAP, CW)           # key-column groups (phase A exps)
    NKG = len(KGL)

    nc = bacc.Bacc(
        "TRN2",
        target_bir_lowering=False,
        debug=False,
        enable_asserts=False,
        num_devices=NCORES,
    )

    qT_d = nc.dram_tensor("qT", (B, E, T_), f32r, kind="ExternalInput").ap()
    qTc_d = nc.dram_tensor("qTc", (B, E, KCAP), f32r, kind="ExternalInput").ap()
    wq_d = nc.dram_tensor("wq", (E, 128), f32r, kind="ExternalInput").ap()
    wk_d = nc.dram_tensor("wk", (E, 128), f32r, kind="ExternalInput").ap()
    wv_d = nc.dram_tensor("wv", (E, 128), f32r, kind="ExternalInput").ap()
    bq_d = nc.dram_tensor("bq", (128, 1), f32, kind="ExternalInput").ap()
    bk_d = nc.dram_tensor("bk", (128, 1), f32, kind="ExternalInput").ap()
    bv_d = nc.dram_tensor("bv", (128, 1), f32, kind="ExternalInput").ap()
    mrow_d = nc.dram_tensor("mrow", (B, KCAP), f32r, kind="ExternalInput").ap()
    mcol_d = nc.dram_tensor("mcol", (128, B, KCN), f32, kind="ExternalInput").ap()
    wout_d = nc.dram_tensor("wout", (64, 2, E), f32r, kind="ExternalInput").ap()
    attn_d = nc.dram_tensor(
        "attn", (B, HPC, T_, KCAP), f32, kind="ExternalOutput"
    ).ap()
    outp_d = nc.dram_tensor("outp", (B, T_, E), f32, kind="ExternalOutput").ap()

    def r(ap):  # operands already declared fp32r
        return ap

    with tile.TileContext(nc) as tc, ExitStack() as ctx:
        consts = ctx.enter_context(tc.tile_pool(name="consts", bufs=1))
        qpool = ctx.enter_context(tc.tile_pool(name="qts", bufs=4))
        sc = ctx.enter_context(tc.tile_pool(name="sc", bufs=2, space="PSUM"))
        pp = ctx.enter_context(tc.tile_pool(name="pp", bufs=2, space="PSUM"))
        po = ctx.enter_context(tc.tile_pool(name="po", bufs=1, space="PSUM"))
        ubp = ctx.enter_context(tc.tile_pool(name="ub", bufs=3))
        atp = ctx.enter_context(tc.tile_pool(name="at", bufs=4))
        hb = ctx.enter_context(tc.tile_pool(name="hb", bufs=1))   # per-b persistents
        zp_pool = ctx.enter_context(tc.tile_pool(name="zs", bufs=4))
        opool = ctx.enter_context(tc.tile_pool(name="op", bufs=4))
        tpool = ctx.enter_context(tc.tile_pool(name="tp", bufs=2))

        # ---- constants ----
        wq_sb = consts.tile([128, KS, 128], f32r, tag="wq")
        wk_sb = consts.tile([128, KS, 128], f32r, tag="wk")
        wv_sb = consts.tile([128, KS, 128], f32r, tag="wv")
        nc.sync.dma_start(wq_sb, wq_d.rearrange("(s p) m -> p s m", p=128))
        nc.sync.dma_start(wk_sb, wk_d.rearrange("(s p) m -> p s m", p=128))
        nc.sync.dma_start(wv_sb, wv_d.rearrange("(s p) m -> p s m", p=128))
        bq_sb = consts.tile([128, 1], f32, tag="bq")
        bk_sb = consts.tile([128, 1], f32, tag="bk")
        bv_sb = consts.tile([128, 1], f32, tag="bv")
        nc.sync.dma_start(bq_sb, bq_d)
        nc.sync.dma_start(bk_sb, bk_d)
        nc.sync.dma_start(bv_sb, bv_d)
        mcol_sb = consts.tile([128, B, KCN], f32, tag="mcol")
        nc.sync.dma_start(mcol_sb, mcol_d)
        wout_sb = consts.tile([64, 2, E], f32r, tag="wout")
        nc.sync.dma_start(wout_sb, wout_d)
        ident0 = consts.tile([128, 128], f32, tag="ident0")
        make_identity(nc, ident0)
        ident = consts.tile([128, 128], f32r, tag="ident")
        nc.vector.tensor_copy(ident, ident0)
        ones0 = consts.tile([1, T_], f32, tag="ones0")
        nc.vector.memset(ones0, 1.0)

        for b in range(B):
            # ---- per-b persistent tiles ----
            qA = [hb.tile([65, T_], f32r, tag=f"qA{hh}", name=f"qA{hh}") for hh in range(2)]
            kA = [hb.tile([65, KCAP], f32r, tag=f"kA{hh}", name=f"kA{hh}") for hh in range(2)]
            vT_sb = hb.tile([128, KCAP], f32r, tag="vT")
            vh = hb.tile([128, KCN, 128], f32r, tag="vh")
            outT = [hb.tile([64, T_], f32r, tag=f"oT{hh}", name=f"oT{hh}") for hh in range(2)]
            for hh in range(2):
                nc.vector.tensor_copy(qA[hh][64:65, :], ones0)
                nc.sync.dma_start(kA[hh][64:65, :], mrow_d[b : b + 1, :])

            # K/V side: compacted key positions from qTc.
            for kco, kcw in _col_chunks(KCAP, 512):
                ksl = slice(kco, kco + kcw)
                qts_l = []
                for ks in range(KS):
                    qts = qpool.tile(
                        [128, 512], f32r, tag="qt", name="qts", bufs=16
                    )
                    nc.sync.dma_start(
                        qts[:, :kcw], qTc_d[b, ks * 128 : (ks + 1) * 128, ksl]
                    )
                    qts_l.append(qts)
                for wsb, dst, bias in ((wk_sb, kA, bk_sb), (wv_sb, None, bv_sb)):
                    ps = pp.tile([128, 512], f32, tag="pp", name="psprk")
                    for ks in range(KS):
                        nc.tensor.matmul(
                            ps[:, :kcw],
                            lhsT=r(wsb[:, ks, :]),
                            rhs=r(qts_l[ks][:, :kcw]),
                            start=(ks == 0),
                            stop=(ks == KS - 1),
                            skip_group_check=True,
                        )
                    if dst is None:
                        nc.vector.tensor_scalar_add(
                            vT_sb[:, ksl], ps[:, :kcw], bias[:, 0:1]
                        )
                    else:
                        nc.vector.tensor_scalar_add(
                            dst[0][0:64, ksl], ps[0:64, :kcw], bias[0:64, 0:1]
                        )
                        shtmp = tpool.tile(
                            [128, 512], f32r, tag="sh", name="shtmp"
                        )
                        nc.vector.tensor_scalar_add(
                            shtmp[64:128, :kcw], ps[64:128, :kcw], bias[64:128, 0:1]
                        )
                        nc.sync.dma_start(dst[1][0:64, ksl], shtmp[64:128, :kcw])

            # ---- projection (pass-sequential, one psum slot at a time) ----
            # Q side: all T query positions from qT.
            for tco, tcw in _col_chunks(T_, 512):
                tsl = slice(tco, tco + tcw)
                qts_l = []
                for ks in range(KS):
                    qts = qpool.tile(
                        [128, 512], f32r, tag="qt", name="qts", bufs=16
                    )
                    nc.sync.dma_start(
                        qts[:, :tcw], qT_d[b, ks * 128 : (ks + 1) * 128, tsl]
                    )
                    qts_l.append(qts)
                ps = pp.tile([128, 512], f32, tag="pp", name="psprq")
                for ks in range(KS):
                    nc.tensor.matmul(
                        ps[:, :tcw],
                        lhsT=r(wq_sb[:, ks, :]),
                        rhs=r(qts_l[ks][:, :tcw]),
                        start=(ks == 0),
                        stop=(ks == KS - 1),
                        skip_group_check=True,
                    )
                nc.vector.tensor_scalar_add(
                    qA[0][0:64, tsl], ps[0:64, :tcw], bq_sb[0:64, 0:1]
                )
                shtmp = tpool.tile([128, 512], f32r, tag="sh", name="shtmp")
                nc.vector.tensor_scalar_add(
                    shtmp[64:128, :tcw], ps[64:128, :tcw], bq_sb[64:128, 0:1]
                )
                nc.sync.dma_start(qA[1][0:64, tsl], shtmp[64:128, :tcw])

            # ---- vh = vT.T per 128-chunk (PE transpose) ----
            for kc in range(KCN):
                pst = pp.tile([128, 128], f32r, tag="pp", name="pst")
                nc.tensor.transpose(pst, vT_sb[:, kc * 128 : (kc + 1) * 128], ident)
                nc.vector.tensor_copy(vh[:, kc, :], pst)

            rz_h = [None, None]

            def phaseAB(hh):
                """Interleaved emission of phase A (attn rows: DMA-heavy) and
                phase B (U_B @ V: ACT/PE-heavy) so ACT and DMA co-saturate."""
                zps = zp_pool.tile(
                    [128, TC, NKG], f32, tag=f"zp{hh}", name=f"zp{hh}", bufs=2
                )
                rz = zp_pool.tile(
                    [128, TC], f32, tag=f"rz{hh}", name=f"rz{hh}", bufs=2
                )
                rz_h[hh] = rz
                otp_l = [None] * NCG

                def b_unit(j):
                    half, kc = divmod(j, KCN)
                    if kc == 0:
                        otp_l[half] = po.tile([64, CW], f32, tag="po", name="otp")
                    otp = otp_l[half]
                    scb = sc.tile([128, CW], f32, tag="sc", name="scb")
                    for o, w in _col_chunks(CW, 512):
                        nc.tensor.matmul(
                            scb[:, o : o + w],
                            lhsT=r(kA[hh][0:64, kc * 128 : (kc + 1) * 128]),
                            rhs=r(qA[hh][0:64, half * CW + o : half * CW + o + w]),
                            start=True,
                            stop=True,
                        )
                    ubt = ubp.tile([128, CW], f32r, tag="ub", name="ubt")
                    nc.scalar.activation(
                        ubt, scb, Exp, bias=mcol_sb[:, b, kc : kc + 1], scale=0.125
                    )
                    for o, w in _col_chunks(CW, 512):
                        nc.tensor.matmul(
                            otp[:, o : o + w],
                            lhsT=r(vh[:, kc, hh * 64 : (hh + 1) * 64]),
                            rhs=r(ubt[:, o : o + w]),
                            start=(kc == 0),
                            stop=(kc == KCN - 1),
                            skip_group_check=True,
                        )
                    if kc == KCN - 1:
                        nc.vector.tensor_copy(
                            outT[hh][:, half * CW : (half + 1) * CW], otp
                        )

                def a_unit(tci):
                    tsl = slice(tci * 128, (tci + 1) * 128)
                    att = atp.tile([128, KCAP], f32, tag="at", name="att")
                    for cg, (ko, kw) in enumerate(KGL):
                        sca = sc.tile([128, CW], f32, tag="sc", name="sca")
                        for o, w in _col_chunks(kw, 512):
                            nc.tensor.matmul(
                                sca[:, o : o + w],
                                lhsT=r(qA[hh][:, tsl]),
                                rhs=r(kA[hh][:, ko + o : ko + o + w]),
                                start=True,
                                stop=True,
                            )
                        nc.scalar.activation(
                            att[:, ko : ko + kw],
                            sca[:, :kw],
                            Exp,
                            scale=0.125,
                            accum_out=zps[:, tci, cg : cg + 1],
                        )
                    ztmp = zp_pool.tile([128, 1], f32, tag="ztmp", name="ztmp", bufs=3)
                    nc.vector.tensor_reduce(
                        ztmp,
                        zps[:, tci, :],
                        axis=mybir.AxisListType.X,
                        op=mybir.AluOpType.add,
                    )
                    nc.vector.reciprocal(rz[:, tci : tci + 1], ztmp)
                    nc.vector.tensor_scalar_mul(att, att, rz[:, tci : tci + 1])
                    nc.sync.dma_start(attn_d[b, hh, tsl, :], att)

                nb = NCG * KCN
                jprev = 0
                for tci in range(TC):
                    a_unit(tci)
                    jnext = (tci + 1) * nb // TC
                    for j in range(jprev, jnext):
                        b_unit(j)
                    jprev = jnext

            def outproj():
                for tci in range(TC):
                    tsl = slice(tci * 128, (tci + 1) * 128)
                    pss = []
                    for hh in range(2):
                        ps = sc.tile([128, E], f32, tag="sc", name="psop")
                        for eo, ew in _col_chunks(E, 512):
                            nc.tensor.matmul(
                                ps[:, eo : eo + ew],
                                lhsT=r(outT[hh][:, tsl]),
                                rhs=r(wout_sb[:, hh, eo : eo + ew]),
                                start=True,
                                stop=True,
                            )
                        pss.append(ps)
                    t0 = tpool.tile([128, E], f32, tag="tp", name="t0")
                    nc.scalar.mul(t0, pss[0], rz_h[0][:, tci : tci + 1])
                    ot = opool.tile([128, E], f32, tag="op", name="ot")
                    nc.scalar.mul(ot, pss[1], rz_h[1][:, tci : tci + 1])
                    nc.vector.tensor_add(ot, ot, t0)
                    nc.sync.dma_start(outp_d[b, tsl, :], ot)

            phaseAB(0)
            phaseAB(1)
            outproj()

    nc.compile()
    return nc


def _key_compaction(mask, T_):
    """Per-batch unmasked key indices, padded to a common multiple of 128."""
    idxs = [np.where(~mask[b])[0] for b in range(B)]
    kmax = max(1, max(len(i) for i in idxs))
    KCAP = min(T_, ((kmax + 127) // 128) * 128)
    pidxs = []
    for i in idxs:
        pad = np.zeros(KCAP - len(i), dtype=i.dtype)  # padded lanes masked out
        pidxs.append(np.concatenate([i, pad]))
    return idxs, pidxs, KCAP


def make_in_maps(q, mask, Wqkv, bqkv, Wout, T_=T):
    """Per-core input dicts (host-side sharding by heads + key compaction)."""
    q = np.ascontiguousarray(np.asarray(q, dtype=np.float32))
    mask = np.asarray(mask).astype(bool)
    Wqkv = np.asarray(Wqkv, dtype=np.float32)
    bqkv = np.asarray(bqkv, dtype=np.float32)
    Wout = np.asarray(Wout, dtype=np.float32)

    idxs, pidxs, KCAP = _key_compaction(mask, T_)
    KCN = KCAP // 128
    qT = np.ascontiguousarray(q.transpose(0, 2, 1))  # [B, E, T]
    qTc = np.stack([qT[b][:, pidxs[b]] for b in range(B)])  # [B, E, KCAP]
    # padding lanes (j >= len(idxs[b])) are masked; real lanes are unmasked
    mrow = np.zeros((B, KCAP), np.float32)
    mcolv = np.zeros((B, KCAP), np.float32)
    for b in range(B):
        mrow[b, len(idxs[b]) :] = MASK_ROW
        mcolv[b, len(idxs[b]) :] = MASK_COL
    mcol = np.ascontiguousarray(mcolv.reshape(B, KCN, 128).transpose(2, 0, 1))

    in_maps = []
    for c in range(NCORES):
        hs = [HPC * c + i for i in range(HPC)]
        wq = np.concatenate([Wqkv[:, h * D : (h + 1) * D] for h in hs], axis=1)
        wk = np.concatenate(
            [Wqkv[:, E + h * D : E + (h + 1) * D] for h in hs], axis=1
        )
        wv = np.concatenate(
            [Wqkv[:, 2 * E + h * D : 2 * E + (h + 1) * D] for h in hs], axis=1
        )
        bq = np.concatenate([bqkv[h * D : (h + 1) * D] for h in hs]).reshape(128, 1)
        bk = np.concatenate(
            [bqkv[E + h * D : E + (h + 1) * D] for h in hs]
        ).reshape(128, 1)
        bv = np.concatenate([bqkv[2 * E + h * D : 2 * E + (h + 1) * D] for h in hs])
        wout = np.stack([Wout[h * D : (h + 1) * D, :] for h in hs], axis=1)
        in_maps.append(
            dict(
                qT=qT,
                qTc=np.ascontiguousarray(qTc),
                wq=np.ascontiguousarray(wq),
                wk=np.ascontiguousarray(wk),
                wv=np.ascontiguousarray(wv),
                bq=np.ascontiguousarray(bq),
                bk=np.ascontiguousarray(bk),
                bv=np.ascontiguousarray(bv.reshape(128, 1)),
                mrow=mrow,
                mcol=mcol,
                wout=np.ascontiguousarray(wout),
            )
        )
    return in_maps, idxs, KCAP


def assemble(results, bout, idxs, T_=T):
    """Combine per-core results into (out, attn), scattering compacted keys."""
    bout = np.asarray(bout, dtype=np.float32)
    attn = np.zeros((B, H, T_, T_), dtype=np.float32)
    out = np.zeros((B, T_, E), dtype=np.float32)
    for c, res in enumerate(results):
        a = res["attn"]  # [B, HPC, T, KCAP]
        for bb in range(B):
            n = len(idxs[bb])
            for i in range(HPC):
                attn[bb, HPC * c + i][:, idxs[bb]] = a[bb, i][:, :n]
        out += res["outp"]
    out += bout
    return out, attn


_CACHE = {}


def _get_nc(KCAP=T):
    key = ("nc", KCAP)
    if key not in _CACHE:
        _CACHE[key] = build_nc(T, KCAP)
    return _CACHE[key]


def kernel(q, k=None, v=None, mask=None, Wqkv=None, bqkv=None, Wout=None, bout=None):
    """Full-input entry point. k/v are unused (reference derives qkv from q)."""
    from concourse.bass_utils import run_bass_kernel_spmd

    in_maps, idxs, KCAP = make_in_maps(q, mask, Wqkv, bqkv, Wout, T_=T)
    nc = _get_nc(KCAP)
    res = run_bass_kernel_spmd(nc, in_maps, core_ids=list(range(NCORES)))
    return assemble(res.results, bout, idxs, T_=T)
